# revision 1
# baseline (speedup 1.0000x reference)
"""GridRNN Trainium2 kernel.

Problem: 2-D grid RNN, B=4, S=T=128, H=256, D=3 depths.
  hx[d][b,i,j] = tanh(xin @ Wx_ih[d].T + bx_ih[d] + hx[d][b,i-1,(j-1)%T] @ Wx_hh[d].T + bx_hh[d])
  hy[d][b,i,j] = tanh(yin @ Wy_ih[d].T + by_ih[d] + hy[d][b,i,j-1]     @ Wy_hh[d].T + by_hh[d])
  (xin/yin = src/trg broadcast at d=0, previous depth's hx/hy for d>0)
  out = stack([hx[D-1], hy[D-1]], axis=-2)   # [B,S,T,2,H]

Key structure: the x-chain and y-chain never mix across depths -> 8 cores =
4 batches x 2 chains.  The x-chain's diagonal dependence hx[i-1,(j-1)%T] is
removed by shearing: u_i[c] = hx[i,(i+c)%T] turns it into a plain carry
u_{i-1}[c], identical in form to the y-chain.  One SPMD program runs on all
8 cores; only the input data (seed, weights) differs per core.  The host
unshears the x outputs and transposes the y outputs.

On-chip layout per step: state u kept as [128(part)=H%128, 2(k=H/128), V=128]
(H on partitions as two k-tiles).  Recurrence out[Hout,V] = W.T tiles (lhsT)
@ state tiles (rhs), accumulated in PSUM, tanh via ScalarE with per-partition
bias.  Depth-0's input term depends only on the step index, so it is
precomputed once as columns and folded into the tanh bias.

This walrus build allows only ONE sync-wait per hardware instruction, so the
kernel is structured to keep Tile's emitted waits at <=1 everywhere: all
constants arrive in a single packed DMA ("blob"), absorber ops fold DMA-queue
semaphores into each engine's vector clock, outputs accumulate in one big
SBUF tile and leave in two large DMAs (few DMA lanes -> short tail drain).
"""

import numpy as np

import concourse.bass as bass
import concourse.tile as tile
from concourse import mybir
from concourse.bass_utils import run_bass_kernel_spmd

B, S, T, H, D = 4, 128, 128, 256, 3
P = 128          # partitions
K = H // P       # 2 k-tiles of H on partitions
F32 = mybir.dt.float32
TANH = mybir.ActivationFunctionType.Tanh

# blob column layout (fp32 words per partition)
W0 = 0                    # wihT: (d, k, m) -> W0 + (d*K+k)*H + m
W1 = W0 + D * K * H       # whhT
B0 = W1 + D * K * H       # bias cols: (d, m) -> B0 + d*K + m
S0 = B0 + D * K           # seed row (this partition's step row)
I0 = S0 + H               # identity
CW = I0 + P

OCHUNK = 64

_cache = {}


def _patched_drain_and_barrier(self, tick_clock, wait_clock):
    """Replacement for TileContext._drain_and_barrier.

    This walrus build lowers at most ONE sync-wait per instruction; the stock
    tail drain carries one wait per active proc.  Semantically the waits only
    need to complete before the final barrier's semaphore cleanup, so spread
    them over single-wait NOPs on the sync engine after the drain.
    """
    drain_inst = self.nc.sync.drain()
    wait_clock.add_sem_waits(
        drain_inst.ins, tile.ScopedClock({None: tick_clock.global_clock})
    )
    ins = drain_inst.ins
    si = ins.sync_info
    if si is not None and len(si.on_wait) > 1:
        waits = list(si.on_wait)
        ins.sync_info = mybir.SyncInfo(on_wait=[waits[0]],
                                       on_update=list(si.on_update))
        for w in waits[1:]:
            nop = self.nc.sync.nop(nofuse=True)
            nop.ins.sync_info = mybir.SyncInfo(on_wait=[w], on_update=[])

    self.nc.all_engine_barrier()
    assert self.sems is not None
    popped = self.nc._tile_sem_poison_stack.pop()
    assert popped is self._sem_poison
    self.nc.clear_and_free_semaphores(list(self.sems.allocated().values()))
    self.nc.all_engine_barrier()


tile.TileContext._drain_and_barrier = _patched_drain_and_barrier


def _build():
    nc = bass.Bass(trn_type="TRN2")

    blob = nc.dram_tensor("blob", [P, CW], F32, kind="ExternalInput")
    # DRAM layout mirrors SBUF exactly ([p, s, k, v]) so the output DMA is
    # 128 fully-contiguous 64KB runs; the host reassembles H = k*128+p.
    out = nc.dram_tensor("out", [P, S, K, T], F32, kind="ExternalOutput")
    out_c = out[:, :, :, :]

    with tile.TileContext(nc) as tc:
        with (
            tc.tile_pool(name="consts", bufs=1) as consts,
            tc.tile_pool(name="u0p", bufs=4) as u0p,
            tc.tile_pool(name="u1p", bufs=4) as u1p,
            tc.tile_pool(name="ps0", bufs=2, space="PSUM") as ps0p,
            tc.tile_pool(name="ps1", bufs=2, space="PSUM") as ps1p,
            tc.tile_pool(name="ps2", bufs=2, space="PSUM") as ps2p,
            tc.tile_pool(name="psi", bufs=1, space="PSUM") as psip,
        ):
            cb = consts.tile([P, CW], F32)
            nc.gpsimd.dma_start(out=cb, in_=blob[:, :])

            def wih(d, k, m):
                c = W0 + (d * K + k) * H + m * P
                return cb[:, c:c + P]

            def whh(d, k, m):
                c = W1 + (d * K + k) * H + m * P
                return cb[:, c:c + P]

            def bias(d, m):
                c = B0 + d * K + m
                return cb[:, c:c + 1]

            seed_sb = cb[:, S0:S0 + H]
            ident = cb[:, I0:I0 + P]

            zeros = consts.tile([P, K, T], F32)
            nc.vector.memset(zeros, 0.0)
            # ScalarE absorber: folds the blob-DMA semaphore into ACT's clock
            scr = consts.tile([P, 4], F32)
            nc.scalar.copy(out=scr[:, 0:1], in_=bias(0, 0))
            # PE absorber + warmup: folds the blob-DMA semaphore into PE's clock
            dummy = psip.tile([32, 32], F32, tag="init")
            nc.tensor.matmul(dummy[:, :], lhsT=cb[0:32, 0:32], rhs=cb[0:32, 0:32],
                             start=True, stop=True)

            # ---- seedT[k] = seed[:, k*128:(k+1)*128].T  (PE transpose)
            seedT_sb = consts.tile([P, K, S], F32)
            pst = psip.tile([P, K, S], F32, tag="init")
            for k in range(K):
                nc.tensor.transpose(pst[:, k, :], seed_sb[:, k * P:(k + 1) * P], ident)
            nc.vector.tensor_copy(seedT_sb, pst)

            # ---- pre0[:, m, s] = (W_ih[0] @ seed[s] + bsum[0])[m*128+p]
            pre0_sb = consts.tile([P, K, S], F32)
            psp = psip.tile([P, K, S], F32, tag="init2")
            for m in range(K):
                for k in range(K):
                    nc.tensor.matmul(
                        psp[:, m, :], lhsT=wih(0, k, m), rhs=seedT_sb[:, k, :],
                        start=(k == 0), stop=(k == K - 1))
            for m in range(K):
                nc.scalar.activation(
                    pre0_sb[:, m, :], psp[:, m, :],
                    mybir.ActivationFunctionType.Identity, bias=bias(0, m))

            # ---- main wavefront: tick t runs d0 step t, d1 step t-1, d2 step t-2
            # All d2 outputs accumulate in one big SBUF tile, leaving in a few
            # large SWDGE DMAs (few DMA lanes keeps the tail drain legal).
            u2all = consts.tile([P, S, K, T], F32)
            u0, u1 = {}, {}
            u0[-1] = zeros
            u1[-1] = zeros

            def rec_mms(ps, d, u_in, u_prev):
                """ps[:,m,:] = (Wih[d] @ u_in + Whh[d] @ u_prev) tiles."""
                for m in range(K):
                    first = True
                    if u_in is not None:
                        for k in range(K):
                            nc.tensor.matmul(ps[:, m, :], lhsT=wih(d, k, m),
                                             rhs=u_in[:, k, :],
                                             start=first, stop=False)
                            first = False
                    for k in range(K):
                        nc.tensor.matmul(ps[:, m, :], lhsT=whh(d, k, m),
                                         rhs=u_prev[:, k, :],
                                         start=first, stop=(k == K - 1))
                        first = False

            for t in range(S + 2):
                if t < S:
                    s = t
                    ps = ps0p.tile([P, K, T], F32, tag="ps0")
                    rec_mms(ps, 0, None, u0[s - 1])
                    u = u0p.tile([P, K, T], F32, tag="u0")
                    for m in range(K):
                        nc.scalar.activation(u[:, m, :], ps[:, m, :], TANH,
                                             bias=pre0_sb[:, m, s:s + 1])
                    u0[s] = u
                if 1 <= t <= S:
                    s = t - 1
                    ps = ps1p.tile([P, K, T], F32, tag="ps1")
                    rec_mms(ps, 1, u0[s], u1[s - 1])
                    u = u1p.tile([P, K, T], F32, tag="u1")
                    for m in range(K):
                        nc.scalar.activation(u[:, m, :], ps[:, m, :], TANH,
                                             bias=bias(1, m))
                    u1[s] = u
                if 2 <= t:
                    s = t - 2
                    ps = ps2p.tile([P, K, T], F32, tag="ps2")
                    u2_prev = zeros if s == 0 else u2all[:, s - 1, :, :]
                    rec_mms(ps, 2, u1[s], u2_prev)
                    for m in range(K):
                        nc.scalar.activation(u2all[:, s, m, :], ps[:, m, :], TANH,
                                             bias=bias(2, m))
                    if (s + 1) % OCHUNK == 0:
                        s0 = s + 1 - OCHUNK
                        nc.gpsimd.dma_start(
                            out=out_c[:, s0:s0 + OCHUNK, :, :],
                            in_=u2all[:, s0:s0 + OCHUNK, :, :])
                for dd in (u0, u1):
                    dd.pop(t - 4, None)

    return nc


def _blob(seed, wT_ih, wT_hh, bs):
    """Pack per-core constants into the [P, CW] blob."""
    b = np.empty((P, CW), np.float32)
    # wihT[d, k*128+p, m] -> cols (d*K+k)*H + m
    b[:, W0:W0 + D * K * H] = (
        wT_ih.reshape(D, K, P, H).transpose(2, 0, 1, 3).reshape(P, D * K * H))
    b[:, W1:W1 + D * K * H] = (
        wT_hh.reshape(D, K, P, H).transpose(2, 0, 1, 3).reshape(P, D * K * H))
    # bias cols: bsum[d, m*128+p] -> col B0 + d*K + m
    b[:, B0:B0 + D * K] = bs.reshape(D, K, P).transpose(2, 0, 1).reshape(P, D * K)
    b[:, S0:S0 + H] = seed
    b[:, I0:I0 + P] = np.eye(P, dtype=np.float32)
    return b


def kernel(src, trg, Wx_ih, Wx_hh, bx_ih, bx_hh, Wy_ih, Wy_hh, by_ih, by_hh):
    if "nc" not in _cache:
        _cache["nc"] = _build()
    nc = _cache["nc"]

    def tr(w):  # [D,H,H] -> W[d].T contiguous
        return np.ascontiguousarray(np.swapaxes(np.asarray(w, np.float32), 1, 2))

    src = np.asarray(src, np.float32)
    trg = np.asarray(trg, np.float32)
    wx_ihT, wx_hhT = tr(Wx_ih), tr(Wx_hh)
    wy_ihT, wy_hhT = tr(Wy_ih), tr(Wy_hh)
    bx = np.asarray(bx_ih, np.float32) + np.asarray(bx_hh, np.float32)
    by = np.asarray(by_ih, np.float32) + np.asarray(by_hh, np.float32)

    in_maps = []
    for b in range(B):  # cores 0-3: x chains
        in_maps.append({"blob": _blob(src[b], wx_ihT, wx_hhT, bx)})
    for b in range(B):  # cores 4-7: y chains
        in_maps.append({"blob": _blob(trg[b], wy_ihT, wy_hhT, by)})

    _cache["last_in_maps"] = in_maps
    globals()["_last_in_maps"] = in_maps
    res = run_bass_kernel_spmd(nc, in_maps, list(range(8)))

    out = np.empty((B, S, T, 2, H), np.float32)
    ii = np.arange(S)[:, None]
    jj = np.arange(T)[None, :]
    idx = (jj - ii) % T  # hx[i,j] = u_i[(j-i)%T]
    for b in range(B):
        # raw core output [p, s, k, v] -> [s, H=k*128+p, v]
        arr = res.results[b]["out"].transpose(1, 2, 0, 3).reshape(S, H, T)
        hx = np.take_along_axis(arr, idx[:, None, :], axis=2)  # [s, H, j]
        out[b, :, :, 0, :] = hx.transpose(0, 2, 1)
        arr = res.results[B + b]["out"].transpose(1, 2, 0, 3).reshape(S, H, T)
        out[b, :, :, 1, :] = arr.transpose(2, 0, 1)  # [j, H, i] -> [i, j, H]
    return out



# revision 7
# speedup vs baseline: 1.3292x; 1.3292x over previous
"""GridRNN Trainium2 kernel (fp16 matmul path).

Problem: 2-D grid RNN, B=4, S=T=128, H=256, D=3 depths.
  hx[d][b,i,j] = tanh(xin @ Wx_ih[d].T + bx_ih[d] + hx[d][b,i-1,(j-1)%T] @ Wx_hh[d].T + bx_hh[d])
  hy[d][b,i,j] = tanh(yin @ Wy_ih[d].T + by_ih[d] + hy[d][b,i,j-1]     @ Wy_hh[d].T + by_hh[d])
  (xin/yin = src/trg broadcast at d=0, previous depth's hx/hy for d>0)
  out = stack([hx[D-1], hy[D-1]], axis=-2)   # [B,S,T,2,H]

8 cores = 4 batches x 2 chains (the x-chain's diagonal dependence is removed
by shearing u_i[c] = hx[i,(i+c)%T], making both chains plain carries).  One
SPMD program; only per-core input data differs.  Host unshears x, transposes y.

Perf design (vs the fp32 baseline at ~616us):
- All matmuls fp16: 1 PE cycle/row instead of fp32's 4.
- Depth-0's input term (W_ih0 @ seed + b) is precomputed on the HOST (fp32)
  and shipped transposed as per-step columns; no on-device seed transpose.
- Biases never ride the activation (the bias AP must be free-size-1, which
  would force 2 acts per depth): instead each depth's PSUM tile is pre-armed
  each tick by a rank-2 PE matmul
      ps[p, m*T+j] = sum_c lhsT[c,p] * ind[c, m*T+j],  ind[c,.] = block c
  with lhsT = the depth's bias pair (pre0[:, s] for depth 0) on partitions
  0..1.  Rec matmuls then accumulate with start=False and ONE fused
  [128, K*T] tanh per depth per tick keeps ScalarE at 3 instrs/tick.
- Tile's vector clocks are NOT transitive across engines and this walrus
  build lowers at most ONE sync-wait per instruction, so every instruction
  may depend on at most ONE foreign engine: arming on the PE makes each
  PSUM tile PE-write-only (WAW free), acts depend only on PE, arming
  matmuls' WAR-vs-act deps are covered by the earlier same-tick rec-matmul
  waits, and the one-time DMA semaphore is absorbed into PE's clock by a
  warmup matmul.  Each PSUM tile owns a full 2KB bank so start=True's lazy
  zeroing (which marks the whole 2KB "zero region") cannot poison others.
"""

import numpy as np

import concourse.bass as bass
import concourse.tile as tile
from concourse import mybir
from concourse.bass_utils import run_bass_kernel_spmd

B, S, T, H, D = 4, 128, 128, 256, 3
P = 128          # partitions
K = H // P       # 2 k-tiles of H on partitions
KT = K * T       # 256: one depth's full output row block
F16 = mybir.dt.float16
F32 = mybir.dt.float32
TANH = mybir.ActivationFunctionType.Tanh

# blob column layout (fp16 words per partition): transposed weights only
WHH0 = 0                     # whhT: (d,k,m) -> WHH0 + (d*K+k)*H + m*P, d=0..2
WIH0 = WHH0 + D * K * H      # wihT: (d,k,m) -> WIH0 + ((d-1)*K+k)*H + m*P, d=1..2
CW = WIH0 + (D - 1) * K * H

# aux tensor layout (2 partitions, fp16): rank-2 arming operands
A_P0T = 0                    # pre0T: A_P0T + s*P + p, s = 0..S-1
A_B12 = A_P0T + S * P        # bias12T: A_B12 + (d-1)*P + p, d = 1..2
A_IND = A_B12 + (D - 1) * P  # indicator [2, KT]: ind[c, m*T+j] = (c == m)
AW = A_IND + KT

OCHUNK = 32

_cache = {}


def _patched_drain_and_barrier(self, tick_clock, wait_clock):
    """Replacement for TileContext._drain_and_barrier.

    This walrus build lowers at most ONE sync-wait per instruction; the stock
    tail drain carries one wait per active proc.  Semantically the waits only
    need to complete before the final barrier's semaphore cleanup, so spread
    them over single-wait NOPs on the sync engine after the drain.
    """
    drain_inst = self.nc.sync.drain()
    wait_clock.add_sem_waits(
        drain_inst.ins, tile.ScopedClock({None: tick_clock.global_clock})
    )
    ins = drain_inst.ins
    si = ins.sync_info
    if si is not None and len(si.on_wait) > 1:
        waits = list(si.on_wait)
        ins.sync_info = mybir.SyncInfo(on_wait=[waits[0]],
                                       on_update=list(si.on_update))
        for w in waits[1:]:
            nop = self.nc.sync.nop(nofuse=True)
            nop.ins.sync_info = mybir.SyncInfo(on_wait=[w], on_update=[])

    self.nc.all_engine_barrier()
    assert self.sems is not None
    popped = self.nc._tile_sem_poison_stack.pop()
    assert popped is self._sem_poison
    self.nc.clear_and_free_semaphores(list(self.sems.allocated().values()))
    self.nc.all_engine_barrier()


tile.TileContext._drain_and_barrier = _patched_drain_and_barrier


def _build():
    nc = bass.Bass(trn_type="TRN2")

    blob = nc.dram_tensor("blob", [P, CW], F16, kind="ExternalInput")
    aux = nc.dram_tensor("aux", [K, AW], F16, kind="ExternalInput")
    # DRAM layout mirrors SBUF exactly ([p, s, kt]); host reassembles H.
    out = nc.dram_tensor("out", [P, S, KT], F16, kind="ExternalOutput")
    out_c = out[:, :, :]

    with tile.TileContext(nc) as tc:
        with (
            tc.tile_pool(name="consts", bufs=1) as consts,
            tc.tile_pool(name="u0p", bufs=4) as u0p,
            tc.tile_pool(name="u1p", bufs=4) as u1p,
            tc.tile_pool(name="ps0", bufs=2, space="PSUM") as ps0p,
            tc.tile_pool(name="ps1", bufs=2, space="PSUM") as ps1p,
            tc.tile_pool(name="ps2", bufs=2, space="PSUM") as ps2p,
            tc.tile_pool(name="psd", bufs=1, space="PSUM") as psdp,
        ):
            cb = consts.tile([P, CW], F16)
            nc.gpsimd.dma_start(out=cb, in_=blob[:, :])
            ax = consts.tile([K, AW], F16)
            nc.gpsimd.dma_start(out=ax, in_=aux[:, :])

            def whh(d, k, m):
                c = WHH0 + (d * K + k) * H + m * P
                return cb[:, c:c + P]

            def wih(d, k, m):
                c = WIH0 + ((d - 1) * K + k) * H + m * P
                return cb[:, c:c + P]

            def arm_lhsT(d, t):
                if d == 0:
                    c = A_P0T + t * P
                else:
                    c = A_B12 + (d - 1) * P
                return ax[:, c:c + P]

            ind = ax[:, A_IND:A_IND + KT]

            zeros = consts.tile([P, KT], F16)
            nc.vector.memset(zeros, 0.0)
            u2all = consts.tile([P, S, KT], F16)

            # PE absorber + warmup: folds the input-DMA queue semaphore into
            # PE's clock (reads ax, the later DMA on the same SWDGE queue).
            # start=True is safe: the dummy owns its full bank.
            dummy = psdp.tile([P, 512], F32, tag="init")
            nc.tensor.matmul(dummy[0:32, 0:32], lhsT=ax[:, 0:32],
                             rhs=ax[:, 0:32], start=True, stop=True)

            def arm_mm(ps, d, t):
                """Rank-2 arming matmul: ps[:, m*T+j] = bias_d[m*128+p]."""
                nc.tensor.matmul(ps[:, 0:KT], lhsT=arm_lhsT(d, t), rhs=ind,
                                 start=True, stop=False, skip_group_check=True)

            def rec_mms(ps, d, u_in, u_prev):
                """ps[:, m*T:(m+1)*T] += (Wih[d] @ u_in + Whh[d] @ u_prev)."""
                per_m = (K if u_in is not None else 0) + K
                n, total = 0, per_m * K
                for m in range(K):
                    o = ps[:, m * T:(m + 1) * T]
                    if u_in is not None:
                        for k in range(K):
                            n += 1
                            nc.tensor.matmul(
                                o, lhsT=wih(d, k, m),
                                rhs=u_in[:, k * T:(k + 1) * T],
                                start=False, stop=(n == total),
                                skip_group_check=True)
                    for k in range(K):
                        n += 1
                        nc.tensor.matmul(
                            o, lhsT=whh(d, k, m),
                            rhs=u_prev[:, k * T:(k + 1) * T],
                            start=False, stop=(n == total),
                            skip_group_check=True)

            u0, u1 = {}, {}
            u0[-1] = zeros
            u1[-1] = zeros

            for t in range(S + 2):
                # ---- depth 0: u0[t] = tanh(pre0[:,t] + Whh0 @ u0[t-1])
                if t < S:
                    ps0 = ps0p.tile([P, 512], F32, tag="ps0")
                    arm_mm(ps0, 0, t)
                    rec_mms(ps0[:, 0:KT], 0, None, u0[t - 1])
                    u = u0p.tile([P, KT], F16, tag="u0")
                    nc.scalar.activation(u, ps0[:, 0:KT], TANH)
                    u0[t] = u

                # ---- depth 1
                if 1 <= t <= S:
                    s = t - 1
                    ps1 = ps1p.tile([P, 512], F32, tag="ps1")
                    arm_mm(ps1, 1, t)
                    rec_mms(ps1[:, 0:KT], 1, u0[s], u1[s - 1])
                    u = u1p.tile([P, KT], F16, tag="u1")
                    nc.scalar.activation(u, ps1[:, 0:KT], TANH)
                    u1[s] = u

                # ---- depth 2 (output depth, accumulated in u2all)
                if 2 <= t:
                    s = t - 2
                    ps2 = ps2p.tile([P, 512], F32, tag="ps2")
                    arm_mm(ps2, 2, t)
                    u2_prev = zeros if s == 0 else u2all[:, s - 1, :]
                    rec_mms(ps2[:, 0:KT], 2, u1[s], u2_prev)
                    nc.scalar.activation(u2all[:, s, :], ps2[:, 0:KT], TANH)
                    if (s + 1) % OCHUNK == 0:
                        c0 = s + 1 - OCHUNK
                        nc.gpsimd.dma_start(
                            out=out_c[:, c0:c0 + OCHUNK, :],
                            in_=u2all[:, c0:c0 + OCHUNK, :])

                for dd in (u0, u1):
                    dd.pop(t - 4, None)

    _strip_same_engine_waits(nc)
    return nc


_ENG_SEM_PREFIX = {
    mybir.EngineType.Activation: "Activation",
    mybir.EngineType.PE: "PE",
    mybir.EngineType.DVE: "DVE",
    mybir.EngineType.Pool: "Pool",
    mybir.EngineType.SP: "SP",
}


def _strip_same_engine_waits(nc):
    """Drop sem waits an instruction holds on its OWN engine's stream sem.

    Tile emits pool-reuse WAW/WAR hazards as explicit sem waits even when
    producer and consumer share an engine; same-engine execution is in-order
    so such waits are provably satisfied at issue.  Removing them keeps every
    instruction at <=1 sync-wait (a hard limit of this walrus build).
    """
    for fn in nc.m.functions:
        for blk in fn.blocks:
            for inst in blk.instructions:
                si = inst.sync_info
                if si is None or not si.on_wait:
                    continue
                pfx = _ENG_SEM_PREFIX.get(inst.engine)
                if pfx is None:
                    continue
                keep = [w for w in si.on_wait
                        if w.ant_name.rsplit("_", 1)[0] != pfx]
                if len(keep) != len(si.on_wait):
                    inst.sync_info = mybir.SyncInfo(
                        on_wait=keep, on_update=list(si.on_update))
                assert len(keep) <= 1, (
                    f"{inst.name}: {len(keep)} foreign waits remain: "
                    f"{[w.ant_name for w in keep]}")


def _blob(whhT, wihT12):
    """Pack per-core transposed weights into the [P, CW] fp16 blob."""
    b = np.empty((P, CW), np.float16)
    b[:, WHH0:WHH0 + D * K * H] = (
        whhT.reshape(D, K, P, H).transpose(2, 0, 1, 3).reshape(P, D * K * H))
    b[:, WIH0:WIH0 + (D - 1) * K * H] = (
        wihT12.reshape(D - 1, K, P, H).transpose(2, 0, 1, 3)
        .reshape(P, (D - 1) * K * H))
    return b


def _aux(W0, seed, b0, bias12):
    """Rank-2 arming operands on partitions 0..1: pre0T, bias12T, indicator."""
    a = np.zeros((K, AW), np.float16)
    # pre0[h, s] = (W0 @ seed.T + b0)[h]; a[c, s*P+p] = pre0[c*128+p, s]
    pre0 = W0 @ seed.T + b0[:, None]
    a[:, A_P0T:A_P0T + S * P] = (
        pre0.reshape(K, P, S).transpose(0, 2, 1).reshape(K, S * P))
    # a[c, (d-1)*P+p] = bias12[d-1, c*128+p]
    a[:, A_B12:A_B12 + (D - 1) * P] = (
        bias12.reshape(D - 1, K, P).transpose(1, 0, 2).reshape(K, (D - 1) * P))
    for c in range(K):
        a[c, A_IND + c * T:A_IND + (c + 1) * T] = 1.0
    return a


def kernel(src, trg, Wx_ih, Wx_hh, bx_ih, bx_hh, Wy_ih, Wy_hh, by_ih, by_hh):
    if "nc" not in _cache:
        _cache["nc"] = _build()
    nc = _cache["nc"]

    def tr(w):  # [D,H,H] -> W[d].T contiguous, fp16
        return np.ascontiguousarray(
            np.swapaxes(np.asarray(w, np.float32), 1, 2)).astype(np.float16)

    src = np.asarray(src, np.float32)
    trg = np.asarray(trg, np.float32)
    blob_x = _blob(tr(Wx_hh), tr(Wx_ih)[1:])
    blob_y = _blob(tr(Wy_hh), tr(Wy_ih)[1:])
    bx = np.asarray(bx_ih, np.float32) + np.asarray(bx_hh, np.float32)
    by = np.asarray(by_ih, np.float32) + np.asarray(by_hh, np.float32)
    Wx0 = np.asarray(Wx_ih, np.float32)[0]
    Wy0 = np.asarray(Wy_ih, np.float32)[0]

    in_maps = []
    for b in range(B):  # cores 0-3: x chains
        in_maps.append({"blob": blob_x,
                        "aux": _aux(Wx0, src[b], bx[0], bx[1:])})
    for b in range(B):  # cores 4-7: y chains
        in_maps.append({"blob": blob_y,
                        "aux": _aux(Wy0, trg[b], by[0], by[1:])})

    _cache["last_in_maps"] = in_maps
    globals()["_last_in_maps"] = in_maps
    res = run_bass_kernel_spmd(nc, in_maps, list(range(8)))

    out = np.empty((B, S, T, 2, H), np.float32)
    ii = np.arange(S)[:, None]
    jj = np.arange(T)[None, :]
    idx = (jj - ii) % T  # hx[i,j] = u_i[(j-i)%T]
    for b in range(B):
        # raw core output [p, s, k*T+v] -> [s, H=k*128+p, v]
        arr = np.asarray(res.results[b]["out"], np.float32)
        arr = arr.reshape(P, S, K, T).transpose(1, 2, 0, 3).reshape(S, H, T)
        hx = np.take_along_axis(arr, idx[:, None, :], axis=2)  # [s, H, j]
        out[b, :, :, 0, :] = hx.transpose(0, 2, 1)
        arr = np.asarray(res.results[B + b]["out"], np.float32)
        arr = arr.reshape(P, S, K, T).transpose(1, 2, 0, 3).reshape(S, H, T)
        out[b, :, :, 1, :] = arr.transpose(2, 0, 1)  # [j, H, i] -> [i, j, H]
    return out


# revision 11
# speedup vs baseline: 1.3305x; 1.0010x over previous
"""GridRNN Trainium2 kernel (fp16 matmul path).

Problem: 2-D grid RNN, B=4, S=T=128, H=256, D=3 depths.
  hx[d][b,i,j] = tanh(xin @ Wx_ih[d].T + bx_ih[d] + hx[d][b,i-1,(j-1)%T] @ Wx_hh[d].T + bx_hh[d])
  hy[d][b,i,j] = tanh(yin @ Wy_ih[d].T + by_ih[d] + hy[d][b,i,j-1]     @ Wy_hh[d].T + by_hh[d])
  (xin/yin = src/trg broadcast at d=0, previous depth's hx/hy for d>0)
  out = stack([hx[D-1], hy[D-1]], axis=-2)   # [B,S,T,2,H]

8 cores = 4 batches x 2 chains (the x-chain's diagonal dependence is removed
by shearing u_i[c] = hx[i,(i+c)%T], making both chains plain carries).  One
SPMD program; only per-core input data differs.  Host unshears x, transposes y.

Perf design (vs the fp32 baseline at ~616us):
- All matmuls fp16: 1 PE cycle/row instead of fp32's 4.
- Depth-0's input term (W_ih0 @ seed + b) is precomputed on the HOST (fp32)
  and shipped transposed as per-step columns; no on-device seed transpose.
- Biases never ride the activation (the bias AP must be free-size-1, which
  would force 2 acts per depth): instead each depth's PSUM tile is pre-armed
  each tick by a rank-2 PE matmul
      ps[p, m*T+j] = sum_c lhsT[c,p] * ind[c, m*T+j],  ind[c,.] = block c
  with lhsT = the depth's bias pair (pre0[:, s] for depth 0) on partitions
  0..1.  Rec matmuls then accumulate with start=False and ONE fused
  [128, K*T] tanh per depth per tick keeps ScalarE at 3 instrs/tick.
- Tile's vector clocks are NOT transitive across engines and this walrus
  build lowers at most ONE sync-wait per instruction, so every instruction
  may depend on at most ONE foreign engine: arming on the PE makes each
  PSUM tile PE-write-only (WAW free), acts depend only on PE, arming
  matmuls' WAR-vs-act deps are covered by the earlier same-tick rec-matmul
  waits, and the one-time DMA semaphore is absorbed into PE's clock by a
  warmup matmul.  Each PSUM tile owns a full 2KB bank so start=True's lazy
  zeroing (which marks the whole 2KB "zero region") cannot poison others.
"""

import numpy as np

import concourse.bass as bass
import concourse.tile as tile
from concourse import mybir
from concourse.bass_utils import run_bass_kernel_spmd

B, S, T, H, D = 4, 128, 128, 256, 3
P = 128          # partitions
K = H // P       # 2 k-tiles of H on partitions
KT = K * T       # 256: one depth's full output row block
F16 = mybir.dt.float16
F32 = mybir.dt.float32
TANH = mybir.ActivationFunctionType.Tanh

# blob column layout (fp16 words per partition): transposed weights only
WHH0 = 0                     # whhT: (d,k,m) -> WHH0 + (d*K+k)*H + m*P, d=0..2
WIH0 = WHH0 + D * K * H      # wihT: (d,k,m) -> WIH0 + ((d-1)*K+k)*H + m*P, d=1..2
CW = WIH0 + (D - 1) * K * H

# aux tensor layout (2 partitions, fp16): rank-2 arming operands
A_P0T = 0                    # pre0T: A_P0T + s*P + p, s = 0..S-1
A_B12 = A_P0T + S * P        # bias12T: A_B12 + (d-1)*P + p, d = 1..2
A_IND = A_B12 + (D - 1) * P  # indicator [2, KT]: ind[c, m*T+j] = (c == m)
AW = A_IND + KT

OCHUNK = 32

_cache = {}


def _patched_drain_and_barrier(self, tick_clock, wait_clock):
    """Replacement for TileContext._drain_and_barrier.

    This walrus build lowers at most ONE sync-wait per instruction; the stock
    tail drain carries one wait per active proc.  Semantically the waits only
    need to complete before the final barrier's semaphore cleanup, so spread
    them over single-wait NOPs on the sync engine after the drain.
    """
    drain_inst = self.nc.sync.drain()
    wait_clock.add_sem_waits(
        drain_inst.ins, tile.ScopedClock({None: tick_clock.global_clock})
    )
    ins = drain_inst.ins
    si = ins.sync_info
    if si is not None and len(si.on_wait) > 1:
        waits = list(si.on_wait)
        ins.sync_info = mybir.SyncInfo(on_wait=[waits[0]],
                                       on_update=list(si.on_update))
        for w in waits[1:]:
            nop = self.nc.sync.nop(nofuse=True)
            nop.ins.sync_info = mybir.SyncInfo(on_wait=[w], on_update=[])

    self.nc.all_engine_barrier()
    assert self.sems is not None
    popped = self.nc._tile_sem_poison_stack.pop()
    assert popped is self._sem_poison
    self.nc.clear_and_free_semaphores(list(self.sems.allocated().values()))
    self.nc.all_engine_barrier()


tile.TileContext._drain_and_barrier = _patched_drain_and_barrier


def _patch_ldw_opt():
    """Compile with walrus --enable-ldw-opt=true.

    Every matmul here carries a fresh stationary (the recurrence cycles 23
    weight tiles per tick), so the separate LDWEIGHTS+MATMUL pairs the
    default pipeline emits serialize the PE (~149ns/pair vs 53ns of math).
    ldw-opt lets walrus overlap/merge the weight loads.
    """
    import concourse.bass_utils as _bu
    if getattr(_bu.run_command, "_ldw_patched", False):
        return
    orig = _bu.run_command

    def run_command(cmd, *a, **kw):
        if LDW_OPT and isinstance(cmd, list):
            cmd = ["--enable-ldw-opt=true" if c == "--enable-ldw-opt=false"
                   else c for c in cmd]
        return orig(cmd, *a, **kw)

    run_command._ldw_patched = True
    _bu.run_command = run_command


LDW_OPT = False
_patch_ldw_opt()


def _build():
    nc = bass.Bass(trn_type="TRN2")

    blob = nc.dram_tensor("blob", [P, CW], F16, kind="ExternalInput")
    aux = nc.dram_tensor("aux", [K, AW], F16, kind="ExternalInput")
    # DRAM layout mirrors SBUF exactly ([p, s, kt]); host reassembles H.
    out = nc.dram_tensor("out", [P, S, KT], F16, kind="ExternalOutput")
    out_c = out[:, :, :]

    with tile.TileContext(nc) as tc:
        with (
            tc.tile_pool(name="consts", bufs=1) as consts,
            tc.tile_pool(name="u0p", bufs=4) as u0p,
            tc.tile_pool(name="u1p", bufs=4) as u1p,
            tc.tile_pool(name="ps0", bufs=2, space="PSUM") as ps0p,
            tc.tile_pool(name="ps1", bufs=2, space="PSUM") as ps1p,
            tc.tile_pool(name="ps2", bufs=2, space="PSUM") as ps2p,
            tc.tile_pool(name="psd", bufs=1, space="PSUM") as psdp,
        ):
            cb = consts.tile([P, CW], F16)
            nc.gpsimd.dma_start(out=cb, in_=blob[:, :])
            ax = consts.tile([K, AW], F16)
            nc.gpsimd.dma_start(out=ax, in_=aux[:, :])

            def whh(d, k, m):
                c = WHH0 + (d * K + k) * H + m * P
                return cb[:, c:c + P]

            def wih(d, k, m):
                c = WIH0 + ((d - 1) * K + k) * H + m * P
                return cb[:, c:c + P]

            def arm_lhsT(d, t):
                if d == 0:
                    c = A_P0T + t * P
                else:
                    c = A_B12 + (d - 1) * P
                return ax[:, c:c + P]

            ind = ax[:, A_IND:A_IND + KT]

            zeros = consts.tile([P, KT], F16)
            nc.vector.memset(zeros, 0.0)
            u2all = consts.tile([P, S, KT], F16)

            # PE absorbers + warmup: the two input DMAs land on different
            # SWDGE queues, so one warmup matmul per tensor folds each DMA
            # semaphore into PE's clock.  start=True is safe: the dummy
            # owns its full bank.
            dummy = psdp.tile([P, 512], F32, tag="init")
            nc.tensor.matmul(dummy[0:32, 0:32], lhsT=cb[0:2, 0:32],
                             rhs=cb[0:2, 0:32], start=True, stop=True)
            nc.tensor.matmul(dummy[0:32, 64:96], lhsT=ax[:, 0:32],
                             rhs=ax[:, 0:32], start=False, stop=True,
                             skip_group_check=True)

            def arm_mm(ps, d, t):
                """Rank-2 arming matmul: ps[:, m*T+j] = bias_d[m*128+p]."""
                nc.tensor.matmul(ps[:, 0:KT], lhsT=arm_lhsT(d, t), rhs=ind,
                                 start=True, stop=False, skip_group_check=True)

            def rec_mms(ps, d, u_in, u_prev):
                """ps[:, m*T:(m+1)*T] += (Wih[d] @ u_in + Whh[d] @ u_prev)."""
                per_m = (K if u_in is not None else 0) + K
                n, total = 0, per_m * K
                for m in range(K):
                    o = ps[:, m * T:(m + 1) * T]
                    if u_in is not None:
                        for k in range(K):
                            n += 1
                            nc.tensor.matmul(
                                o, lhsT=wih(d, k, m),
                                rhs=u_in[:, k * T:(k + 1) * T],
                                start=False, stop=(n == total),
                                skip_group_check=True)
                    for k in range(K):
                        n += 1
                        nc.tensor.matmul(
                            o, lhsT=whh(d, k, m),
                            rhs=u_prev[:, k * T:(k + 1) * T],
                            start=False, stop=(n == total),
                            skip_group_check=True)

            u0, u1 = {}, {}
            u0[-1] = zeros
            u1[-1] = zeros

            for t in range(S + 2):
                # ---- depth 0: u0[t] = tanh(pre0[:,t] + Whh0 @ u0[t-1])
                if t < S:
                    ps0 = ps0p.tile([P, 512], F32, tag="ps0")
                    arm_mm(ps0, 0, t)
                    rec_mms(ps0[:, 0:KT], 0, None, u0[t - 1])
                    u = u0p.tile([P, KT], F16, tag="u0")
                    nc.scalar.activation(u, ps0[:, 0:KT], TANH)
                    u0[t] = u

                # ---- depth 1
                if 1 <= t <= S:
                    s = t - 1
                    ps1 = ps1p.tile([P, 512], F32, tag="ps1")
                    arm_mm(ps1, 1, t)
                    rec_mms(ps1[:, 0:KT], 1, u0[s], u1[s - 1])
                    u = u1p.tile([P, KT], F16, tag="u1")
                    nc.scalar.activation(u, ps1[:, 0:KT], TANH)
                    u1[s] = u

                # ---- depth 2 (output depth, accumulated in u2all)
                if 2 <= t:
                    s = t - 2
                    ps2 = ps2p.tile([P, 512], F32, tag="ps2")
                    arm_mm(ps2, 2, t)
                    u2_prev = zeros if s == 0 else u2all[:, s - 1, :]
                    rec_mms(ps2[:, 0:KT], 2, u1[s], u2_prev)
                    nc.scalar.activation(u2all[:, s, :], ps2[:, 0:KT], TANH)
                    if (s + 1) % OCHUNK == 0:
                        c0 = s + 1 - OCHUNK
                        nc.gpsimd.dma_start(
                            out=out_c[:, c0:c0 + OCHUNK, :],
                            in_=u2all[:, c0:c0 + OCHUNK, :])

                for dd in (u0, u1):
                    dd.pop(t - 4, None)

    _strip_same_engine_waits(nc)
    _fuse_ldweights(nc)
    return nc


def _fuse_ldweights(nc):
    """Convert split LDWEIGHTS+MATMUL pairs back to self-loading matmuls.

    The scheduler splits every 2-byte matmul into an explicit InstLdweights
    followed by the InstMatmult (ldweights=False).  On hardware the pair
    executes serially (~149ns vs 53ns of math for a 128x128 fp16 tile).  The
    split matmul still carries both operands, so dropping the InstLdweights
    and restoring ldweights=None yields the fp32-style self-loading form.
    """
    for fn in nc.m.functions:
        for blk in fn.blocks:
            insts = blk.instructions
            out, pending = [], None
            for inst in insts:
                if type(inst).__name__ == "InstLdweights":
                    assert pending is None
                    pending = inst
                    continue
                if pending is not None:
                    assert type(inst).__name__ == "InstMatmult", inst
                    inst.ldweights = None
                    psi, si = pending.sync_info, inst.sync_info
                    waits = list(psi.on_wait if psi else []) + \
                        list(si.on_wait if si else [])
                    upds = list(psi.on_update if psi else []) + \
                        list(si.on_update if si else [])
                    assert len(waits) <= 1, waits
                    if waits or upds:
                        inst.sync_info = mybir.SyncInfo(
                            on_wait=waits, on_update=upds)
                    pending = None
                out.append(inst)
            assert pending is None
            if len(out) != len(insts):
                blk.instructions = out


_ENG_SEM_PREFIX = {
    mybir.EngineType.Activation: "Activation",
    mybir.EngineType.PE: "PE",
    mybir.EngineType.DVE: "DVE",
    mybir.EngineType.Pool: "Pool",
    mybir.EngineType.SP: "SP",
}


def _strip_same_engine_waits(nc):
    """Drop sem waits an instruction holds on its OWN engine's stream sem.

    Tile emits pool-reuse WAW/WAR hazards as explicit sem waits even when
    producer and consumer share an engine; same-engine execution is in-order
    so such waits are provably satisfied at issue.  Removing them keeps every
    instruction at <=1 sync-wait (a hard limit of this walrus build).
    """
    for fn in nc.m.functions:
        for blk in fn.blocks:
            for inst in blk.instructions:
                si = inst.sync_info
                if si is None or not si.on_wait:
                    continue
                pfx = _ENG_SEM_PREFIX.get(inst.engine)
                if pfx is None:
                    continue
                keep = [w for w in si.on_wait
                        if w.ant_name.rsplit("_", 1)[0] != pfx]
                if len(keep) != len(si.on_wait):
                    inst.sync_info = mybir.SyncInfo(
                        on_wait=keep, on_update=list(si.on_update))
                assert len(keep) <= 1, (
                    f"{inst.name}: {len(keep)} foreign waits remain: "
                    f"{[w.ant_name for w in keep]}")


def _blob(whhT, wihT12):
    """Pack per-core transposed weights into the [P, CW] fp16 blob."""
    b = np.empty((P, CW), np.float16)
    b[:, WHH0:WHH0 + D * K * H] = (
        whhT.reshape(D, K, P, H).transpose(2, 0, 1, 3).reshape(P, D * K * H))
    b[:, WIH0:WIH0 + (D - 1) * K * H] = (
        wihT12.reshape(D - 1, K, P, H).transpose(2, 0, 1, 3)
        .reshape(P, (D - 1) * K * H))
    return b


def _aux(W0, seed, b0, bias12):
    """Rank-2 arming operands on partitions 0..1: pre0T, bias12T, indicator."""
    a = np.zeros((K, AW), np.float16)
    # pre0[h, s] = (W0 @ seed.T + b0)[h]; a[c, s*P+p] = pre0[c*128+p, s]
    pre0 = W0 @ seed.T + b0[:, None]
    a[:, A_P0T:A_P0T + S * P] = (
        pre0.reshape(K, P, S).transpose(0, 2, 1).reshape(K, S * P))
    # a[c, (d-1)*P+p] = bias12[d-1, c*128+p]
    a[:, A_B12:A_B12 + (D - 1) * P] = (
        bias12.reshape(D - 1, K, P).transpose(1, 0, 2).reshape(K, (D - 1) * P))
    for c in range(K):
        a[c, A_IND + c * T:A_IND + (c + 1) * T] = 1.0
    return a


def kernel(src, trg, Wx_ih, Wx_hh, bx_ih, bx_hh, Wy_ih, Wy_hh, by_ih, by_hh):
    if "nc" not in _cache:
        _cache["nc"] = _build()
    nc = _cache["nc"]

    def tr(w):  # [D,H,H] -> W[d].T contiguous, fp16
        return np.ascontiguousarray(
            np.swapaxes(np.asarray(w, np.float32), 1, 2)).astype(np.float16)

    src = np.asarray(src, np.float32)
    trg = np.asarray(trg, np.float32)
    blob_x = _blob(tr(Wx_hh), tr(Wx_ih)[1:])
    blob_y = _blob(tr(Wy_hh), tr(Wy_ih)[1:])
    bx = np.asarray(bx_ih, np.float32) + np.asarray(bx_hh, np.float32)
    by = np.asarray(by_ih, np.float32) + np.asarray(by_hh, np.float32)
    Wx0 = np.asarray(Wx_ih, np.float32)[0]
    Wy0 = np.asarray(Wy_ih, np.float32)[0]

    in_maps = []
    for b in range(B):  # cores 0-3: x chains
        in_maps.append({"blob": blob_x,
                        "aux": _aux(Wx0, src[b], bx[0], bx[1:])})
    for b in range(B):  # cores 4-7: y chains
        in_maps.append({"blob": blob_y,
                        "aux": _aux(Wy0, trg[b], by[0], by[1:])})

    _cache["last_in_maps"] = in_maps
    globals()["_last_in_maps"] = in_maps
    res = run_bass_kernel_spmd(nc, in_maps, list(range(8)))

    out = np.empty((B, S, T, 2, H), np.float32)
    ii = np.arange(S)[:, None]
    jj = np.arange(T)[None, :]
    idx = (jj - ii) % T  # hx[i,j] = u_i[(j-i)%T]
    for b in range(B):
        # raw core output [p, s, k*T+v] -> [s, H=k*128+p, v]
        arr = np.asarray(res.results[b]["out"], np.float32)
        arr = arr.reshape(P, S, K, T).transpose(1, 2, 0, 3).reshape(S, H, T)
        hx = np.take_along_axis(arr, idx[:, None, :], axis=2)  # [s, H, j]
        out[b, :, :, 0, :] = hx.transpose(0, 2, 1)
        arr = np.asarray(res.results[B + b]["out"], np.float32)
        arr = arr.reshape(P, S, K, T).transpose(1, 2, 0, 3).reshape(S, H, T)
        out[b, :, :, 1, :] = arr.transpose(2, 0, 1)  # [j, H, i] -> [i, j, H]
    return out


# revision 12
# speedup vs baseline: 2.1153x; 1.5898x over previous
"""GridRNN Trainium2 kernel (fp16 matmul path).

Problem: 2-D grid RNN, B=4, S=T=128, H=256, D=3 depths.
  hx[d][b,i,j] = tanh(xin @ Wx_ih[d].T + bx_ih[d] + hx[d][b,i-1,(j-1)%T] @ Wx_hh[d].T + bx_hh[d])
  hy[d][b,i,j] = tanh(yin @ Wy_ih[d].T + by_ih[d] + hy[d][b,i,j-1]     @ Wy_hh[d].T + by_hh[d])
  (xin/yin = src/trg broadcast at d=0, previous depth's hx/hy for d>0)
  out = stack([hx[D-1], hy[D-1]], axis=-2)   # [B,S,T,2,H]

8 cores = 4 batches x 2 chains (the x-chain's diagonal dependence is removed
by shearing u_i[c] = hx[i,(i+c)%T], making both chains plain carries).  One
SPMD program; only per-core input data differs.  Host unshears x, transposes y.

Perf design (vs the fp32 baseline at ~616us):
- All matmuls fp16: 1 PE cycle/row instead of fp32's 4.
- Depth-0's input term (W_ih0 @ seed + b) is precomputed on the HOST (fp32)
  and shipped transposed as per-step columns; no on-device seed transpose.
- Biases never ride the activation (the bias AP must be free-size-1, which
  would force 2 acts per depth): instead each depth's PSUM tile is pre-armed
  each tick by a rank-2 PE matmul
      ps[p, m*T+j] = sum_c lhsT[c,p] * ind[c, m*T+j],  ind[c,.] = block c
  with lhsT = the depth's bias pair (pre0[:, s] for depth 0) on partitions
  0..1.  Rec matmuls then accumulate with start=False and ONE fused
  [128, K*T] tanh per depth per tick keeps ScalarE at 3 instrs/tick.
- Tile's vector clocks are NOT transitive across engines and this walrus
  build lowers at most ONE sync-wait per instruction, so every instruction
  may depend on at most ONE foreign engine: arming on the PE makes each
  PSUM tile PE-write-only (WAW free), acts depend only on PE, arming
  matmuls' WAR-vs-act deps are covered by the earlier same-tick rec-matmul
  waits, and the one-time DMA semaphore is absorbed into PE's clock by a
  warmup matmul.  Each PSUM tile owns a full 2KB bank so start=True's lazy
  zeroing (which marks the whole 2KB "zero region") cannot poison others.
"""

import numpy as np

import concourse.bass as bass
import concourse.tile as tile
from concourse import mybir
from concourse.bass_utils import run_bass_kernel_spmd

B, S, T, H, D = 4, 128, 128, 256, 3
P = 128          # partitions
K = H // P       # 2 k-tiles of H on partitions
KT = K * T       # 256: one depth's full output row block
F16 = mybir.dt.float16
F32 = mybir.dt.float32
TANH = mybir.ActivationFunctionType.Tanh

# blob column layout (fp16 words per partition): transposed weights only
WHH0 = 0                     # whhT: (d,k,m) -> WHH0 + (d*K+k)*H + m*P, d=0..2
WIH0 = WHH0 + D * K * H      # wihT: (d,k,m) -> WIH0 + ((d-1)*K+k)*H + m*P, d=1..2
CW = WIH0 + (D - 1) * K * H

# aux tensor layout (2 partitions, fp16): rank-2 arming operands
A_P0T = 0                    # pre0T: A_P0T + s*P + p, s = 0..S-1
A_B12 = A_P0T + S * P        # bias12T: A_B12 + (d-1)*P + p, d = 1..2
A_IND = A_B12 + (D - 1) * P  # indicator [2, KT]: ind[c, m*T+j] = (c == m)
AW = A_IND + KT

OCHUNK = 32

_cache = {}


def _patched_drain_and_barrier(self, tick_clock, wait_clock):
    """Replacement for TileContext._drain_and_barrier.

    This walrus build lowers at most ONE sync-wait per instruction; the stock
    tail drain carries one wait per active proc.  Semantically the waits only
    need to complete before the final barrier's semaphore cleanup, so spread
    them over single-wait NOPs on the sync engine after the drain.
    """
    drain_inst = self.nc.sync.drain()
    wait_clock.add_sem_waits(
        drain_inst.ins, tile.ScopedClock({None: tick_clock.global_clock})
    )
    ins = drain_inst.ins
    si = ins.sync_info
    if si is not None and len(si.on_wait) > 1:
        waits = list(si.on_wait)
        ins.sync_info = mybir.SyncInfo(on_wait=[waits[0]],
                                       on_update=list(si.on_update))
        for w in waits[1:]:
            nop = self.nc.sync.nop(nofuse=True)
            nop.ins.sync_info = mybir.SyncInfo(on_wait=[w], on_update=[])

    self.nc.all_engine_barrier()
    assert self.sems is not None
    popped = self.nc._tile_sem_poison_stack.pop()
    assert popped is self._sem_poison
    self.nc.clear_and_free_semaphores(list(self.sems.allocated().values()))
    self.nc.all_engine_barrier()


tile.TileContext._drain_and_barrier = _patched_drain_and_barrier


def _patch_ldw_opt():
    """Compile with walrus --enable-ldw-opt=true.

    Every matmul here carries a fresh stationary (the recurrence cycles 23
    weight tiles per tick), so the separate LDWEIGHTS+MATMUL pairs the
    default pipeline emits serialize the PE (~149ns/pair vs 53ns of math).
    ldw-opt lets walrus overlap/merge the weight loads.
    """
    import concourse.bass_utils as _bu
    if getattr(_bu.run_command, "_ldw_patched", False):
        return
    orig = _bu.run_command

    def run_command(cmd, *a, **kw):
        if LDW_OPT and isinstance(cmd, list):
            cmd = ["--enable-ldw-opt=true" if c == "--enable-ldw-opt=false"
                   else c for c in cmd]
        return orig(cmd, *a, **kw)

    run_command._ldw_patched = True
    _bu.run_command = run_command


LDW_OPT = True
_patch_ldw_opt()


def _build():
    nc = bass.Bass(trn_type="TRN2")

    blob = nc.dram_tensor("blob", [P, CW], F16, kind="ExternalInput")
    aux = nc.dram_tensor("aux", [K, AW], F16, kind="ExternalInput")
    # DRAM layout mirrors SBUF exactly ([p, s, kt]); host reassembles H.
    out = nc.dram_tensor("out", [P, S, KT], F16, kind="ExternalOutput")
    out_c = out[:, :, :]

    with tile.TileContext(nc) as tc:
        with (
            tc.tile_pool(name="consts", bufs=1) as consts,
            tc.tile_pool(name="u0p", bufs=4) as u0p,
            tc.tile_pool(name="u1p", bufs=4) as u1p,
            tc.tile_pool(name="ps0", bufs=2, space="PSUM") as ps0p,
            tc.tile_pool(name="ps1", bufs=2, space="PSUM") as ps1p,
            tc.tile_pool(name="ps2", bufs=2, space="PSUM") as ps2p,
            tc.tile_pool(name="psd", bufs=1, space="PSUM") as psdp,
        ):
            cb = consts.tile([P, CW], F16)
            nc.gpsimd.dma_start(out=cb, in_=blob[:, :])
            ax = consts.tile([K, AW], F16)
            nc.gpsimd.dma_start(out=ax, in_=aux[:, :])

            def whh(d, k, m):
                c = WHH0 + (d * K + k) * H + m * P
                return cb[:, c:c + P]

            def wih(d, k, m):
                c = WIH0 + ((d - 1) * K + k) * H + m * P
                return cb[:, c:c + P]

            def arm_lhsT(d, t):
                if d == 0:
                    c = A_P0T + t * P
                else:
                    c = A_B12 + (d - 1) * P
                return ax[:, c:c + P]

            ind = ax[:, A_IND:A_IND + KT]

            zeros = consts.tile([P, KT], F16)
            nc.vector.memset(zeros, 0.0)
            u2all = consts.tile([P, S, KT], F16)

            # PE absorbers + warmup: the two input DMAs land on different
            # SWDGE queues, so one warmup matmul per tensor folds each DMA
            # semaphore into PE's clock.  start=True is safe: the dummy
            # owns its full bank.
            dummy = psdp.tile([P, 512], F32, tag="init")
            nc.tensor.matmul(dummy[0:32, 0:32], lhsT=cb[0:2, 0:32],
                             rhs=cb[0:2, 0:32], start=True, stop=True)
            nc.tensor.matmul(dummy[0:32, 64:96], lhsT=ax[:, 0:32],
                             rhs=ax[:, 0:32], start=False, stop=True,
                             skip_group_check=True)

            def arm_mm(ps, d, t):
                """Rank-2 arming matmul: ps[:, m*T+j] = bias_d[m*128+p]."""
                nc.tensor.matmul(ps[:, 0:KT], lhsT=arm_lhsT(d, t), rhs=ind,
                                 start=True, stop=False, skip_group_check=True)

            def rec_mms(ps, d, u_in, u_prev):
                """ps[:, m*T:(m+1)*T] += (Wih[d] @ u_in + Whh[d] @ u_prev)."""
                per_m = (K if u_in is not None else 0) + K
                n, total = 0, per_m * K
                for m in range(K):
                    o = ps[:, m * T:(m + 1) * T]
                    if u_in is not None:
                        for k in range(K):
                            n += 1
                            nc.tensor.matmul(
                                o, lhsT=wih(d, k, m),
                                rhs=u_in[:, k * T:(k + 1) * T],
                                start=False, stop=(n == total),
                                skip_group_check=True)
                    for k in range(K):
                        n += 1
                        nc.tensor.matmul(
                            o, lhsT=whh(d, k, m),
                            rhs=u_prev[:, k * T:(k + 1) * T],
                            start=False, stop=(n == total),
                            skip_group_check=True)

            u0, u1 = {}, {}
            u0[-1] = zeros
            u1[-1] = zeros

            for t in range(S + 2):
                # ---- depth 0: u0[t] = tanh(pre0[:,t] + Whh0 @ u0[t-1])
                if t < S:
                    ps0 = ps0p.tile([P, 512], F32, tag="ps0")
                    arm_mm(ps0, 0, t)
                    rec_mms(ps0[:, 0:KT], 0, None, u0[t - 1])
                    u = u0p.tile([P, KT], F16, tag="u0")
                    nc.scalar.activation(u, ps0[:, 0:KT], TANH)
                    u0[t] = u

                # ---- depth 1
                if 1 <= t <= S:
                    s = t - 1
                    ps1 = ps1p.tile([P, 512], F32, tag="ps1")
                    arm_mm(ps1, 1, t)
                    rec_mms(ps1[:, 0:KT], 1, u0[s], u1[s - 1])
                    u = u1p.tile([P, KT], F16, tag="u1")
                    nc.scalar.activation(u, ps1[:, 0:KT], TANH)
                    u1[s] = u

                # ---- depth 2 (output depth, accumulated in u2all)
                if 2 <= t:
                    s = t - 2
                    ps2 = ps2p.tile([P, 512], F32, tag="ps2")
                    arm_mm(ps2, 2, t)
                    u2_prev = zeros if s == 0 else u2all[:, s - 1, :]
                    rec_mms(ps2[:, 0:KT], 2, u1[s], u2_prev)
                    nc.scalar.activation(u2all[:, s, :], ps2[:, 0:KT], TANH)
                    if (s + 1) % OCHUNK == 0:
                        c0 = s + 1 - OCHUNK
                        nc.gpsimd.dma_start(
                            out=out_c[:, c0:c0 + OCHUNK, :],
                            in_=u2all[:, c0:c0 + OCHUNK, :])

                for dd in (u0, u1):
                    dd.pop(t - 4, None)

    _strip_same_engine_waits(nc)
    _fuse_ldweights(nc)
    return nc


def _fuse_ldweights(nc):
    """Convert split LDWEIGHTS+MATMUL pairs back to self-loading matmuls.

    The scheduler splits every 2-byte matmul into an explicit InstLdweights
    followed by the InstMatmult (ldweights=False).  On hardware the pair
    executes serially (~149ns vs 53ns of math for a 128x128 fp16 tile).  The
    split matmul still carries both operands, so dropping the InstLdweights
    and restoring ldweights=None yields the fp32-style self-loading form.
    """
    for fn in nc.m.functions:
        for blk in fn.blocks:
            insts = blk.instructions
            out, pending = [], None
            for inst in insts:
                if type(inst).__name__ == "InstLdweights":
                    assert pending is None
                    pending = inst
                    continue
                if pending is not None:
                    assert type(inst).__name__ == "InstMatmult", inst
                    inst.ldweights = None
                    psi, si = pending.sync_info, inst.sync_info
                    waits = list(psi.on_wait if psi else []) + \
                        list(si.on_wait if si else [])
                    upds = list(psi.on_update if psi else []) + \
                        list(si.on_update if si else [])
                    assert len(waits) <= 1, waits
                    if waits or upds:
                        inst.sync_info = mybir.SyncInfo(
                            on_wait=waits, on_update=upds)
                    pending = None
                out.append(inst)
            assert pending is None
            if len(out) != len(insts):
                blk.instructions = out


_ENG_SEM_PREFIX = {
    mybir.EngineType.Activation: "Activation",
    mybir.EngineType.PE: "PE",
    mybir.EngineType.DVE: "DVE",
    mybir.EngineType.Pool: "Pool",
    mybir.EngineType.SP: "SP",
}


def _strip_same_engine_waits(nc):
    """Drop sem waits an instruction holds on its OWN engine's stream sem.

    Tile emits pool-reuse WAW/WAR hazards as explicit sem waits even when
    producer and consumer share an engine; same-engine execution is in-order
    so such waits are provably satisfied at issue.  Removing them keeps every
    instruction at <=1 sync-wait (a hard limit of this walrus build).
    """
    for fn in nc.m.functions:
        for blk in fn.blocks:
            for inst in blk.instructions:
                si = inst.sync_info
                if si is None or not si.on_wait:
                    continue
                pfx = _ENG_SEM_PREFIX.get(inst.engine)
                if pfx is None:
                    continue
                keep = [w for w in si.on_wait
                        if w.ant_name.rsplit("_", 1)[0] != pfx]
                if len(keep) != len(si.on_wait):
                    inst.sync_info = mybir.SyncInfo(
                        on_wait=keep, on_update=list(si.on_update))
                assert len(keep) <= 1, (
                    f"{inst.name}: {len(keep)} foreign waits remain: "
                    f"{[w.ant_name for w in keep]}")


def _blob(whhT, wihT12):
    """Pack per-core transposed weights into the [P, CW] fp16 blob."""
    b = np.empty((P, CW), np.float16)
    b[:, WHH0:WHH0 + D * K * H] = (
        whhT.reshape(D, K, P, H).transpose(2, 0, 1, 3).reshape(P, D * K * H))
    b[:, WIH0:WIH0 + (D - 1) * K * H] = (
        wihT12.reshape(D - 1, K, P, H).transpose(2, 0, 1, 3)
        .reshape(P, (D - 1) * K * H))
    return b


def _aux(W0, seed, b0, bias12):
    """Rank-2 arming operands on partitions 0..1: pre0T, bias12T, indicator."""
    a = np.zeros((K, AW), np.float16)
    # pre0[h, s] = (W0 @ seed.T + b0)[h]; a[c, s*P+p] = pre0[c*128+p, s]
    pre0 = W0 @ seed.T + b0[:, None]
    a[:, A_P0T:A_P0T + S * P] = (
        pre0.reshape(K, P, S).transpose(0, 2, 1).reshape(K, S * P))
    # a[c, (d-1)*P+p] = bias12[d-1, c*128+p]
    a[:, A_B12:A_B12 + (D - 1) * P] = (
        bias12.reshape(D - 1, K, P).transpose(1, 0, 2).reshape(K, (D - 1) * P))
    for c in range(K):
        a[c, A_IND + c * T:A_IND + (c + 1) * T] = 1.0
    return a


def kernel(src, trg, Wx_ih, Wx_hh, bx_ih, bx_hh, Wy_ih, Wy_hh, by_ih, by_hh):
    if "nc" not in _cache:
        _cache["nc"] = _build()
    nc = _cache["nc"]

    def tr(w):  # [D,H,H] -> W[d].T contiguous, fp16
        return np.ascontiguousarray(
            np.swapaxes(np.asarray(w, np.float32), 1, 2)).astype(np.float16)

    src = np.asarray(src, np.float32)
    trg = np.asarray(trg, np.float32)
    blob_x = _blob(tr(Wx_hh), tr(Wx_ih)[1:])
    blob_y = _blob(tr(Wy_hh), tr(Wy_ih)[1:])
    bx = np.asarray(bx_ih, np.float32) + np.asarray(bx_hh, np.float32)
    by = np.asarray(by_ih, np.float32) + np.asarray(by_hh, np.float32)
    Wx0 = np.asarray(Wx_ih, np.float32)[0]
    Wy0 = np.asarray(Wy_ih, np.float32)[0]

    in_maps = []
    for b in range(B):  # cores 0-3: x chains
        in_maps.append({"blob": blob_x,
                        "aux": _aux(Wx0, src[b], bx[0], bx[1:])})
    for b in range(B):  # cores 4-7: y chains
        in_maps.append({"blob": blob_y,
                        "aux": _aux(Wy0, trg[b], by[0], by[1:])})

    _cache["last_in_maps"] = in_maps
    globals()["_last_in_maps"] = in_maps
    res = run_bass_kernel_spmd(nc, in_maps, list(range(8)))

    out = np.empty((B, S, T, 2, H), np.float32)
    ii = np.arange(S)[:, None]
    jj = np.arange(T)[None, :]
    idx = (jj - ii) % T  # hx[i,j] = u_i[(j-i)%T]
    for b in range(B):
        # raw core output [p, s, k*T+v] -> [s, H=k*128+p, v]
        arr = np.asarray(res.results[b]["out"], np.float32)
        arr = arr.reshape(P, S, K, T).transpose(1, 2, 0, 3).reshape(S, H, T)
        hx = np.take_along_axis(arr, idx[:, None, :], axis=2)  # [s, H, j]
        out[b, :, :, 0, :] = hx.transpose(0, 2, 1)
        arr = np.asarray(res.results[B + b]["out"], np.float32)
        arr = arr.reshape(P, S, K, T).transpose(1, 2, 0, 3).reshape(S, H, T)
        out[b, :, :, 1, :] = arr.transpose(2, 0, 1)  # [j, H, i] -> [i, j, H]
    return out


# revision 13
# speedup vs baseline: 2.5382x; 1.1999x over previous
"""GridRNN Trainium2 kernel (fp16 matmul path).

Problem: 2-D grid RNN, B=4, S=T=128, H=256, D=3 depths.
  hx[d][b,i,j] = tanh(xin @ Wx_ih[d].T + bx_ih[d] + hx[d][b,i-1,(j-1)%T] @ Wx_hh[d].T + bx_hh[d])
  hy[d][b,i,j] = tanh(yin @ Wy_ih[d].T + by_ih[d] + hy[d][b,i,j-1]     @ Wy_hh[d].T + by_hh[d])
  (xin/yin = src/trg broadcast at d=0, previous depth's hx/hy for d>0)
  out = stack([hx[D-1], hy[D-1]], axis=-2)   # [B,S,T,2,H]

8 cores = 4 batches x 2 chains (the x-chain's diagonal dependence is removed
by shearing u_i[c] = hx[i,(i+c)%T], making both chains plain carries).  One
SPMD program; only per-core input data differs.  Host unshears x, transposes y.

Perf design (vs the fp32 baseline at ~616us):
- All matmuls fp16: 1 PE cycle/row instead of fp32's 4.
- Depth-0's input term (W_ih0 @ seed + b) is precomputed on the HOST (fp32)
  and shipped transposed as per-step columns; no on-device seed transpose.
- Biases never ride the activation (the bias AP must be free-size-1, which
  would force 2 acts per depth): instead each depth's PSUM tile is pre-armed
  each tick by a rank-2 PE matmul
      ps[p, m*T+j] = sum_c lhsT[c,p] * ind[c, m*T+j],  ind[c,.] = block c
  with lhsT = the depth's bias pair (pre0[:, s] for depth 0) on partitions
  0..1.  Rec matmuls then accumulate with start=False and ONE fused
  [128, K*T] tanh per depth per tick keeps ScalarE at 3 instrs/tick.
- Tile's vector clocks are NOT transitive across engines and this walrus
  build lowers at most ONE sync-wait per instruction, so every instruction
  may depend on at most ONE foreign engine: arming on the PE makes each
  PSUM tile PE-write-only (WAW free), acts depend only on PE, arming
  matmuls' WAR-vs-act deps are covered by the earlier same-tick rec-matmul
  waits, and the one-time DMA semaphore is absorbed into PE's clock by a
  warmup matmul.  Each PSUM tile owns a full 2KB bank so start=True's lazy
  zeroing (which marks the whole 2KB "zero region") cannot poison others.
"""

import numpy as np

import concourse.bass as bass
import concourse.tile as tile
from concourse import mybir
from concourse.bass_utils import run_bass_kernel_spmd

B, S, T, H, D = 4, 128, 128, 256, 3
P = 128          # partitions
K = H // P       # 2 k-tiles of H on partitions
KT = K * T       # 256: one depth's full output row block
F16 = mybir.dt.float16
F32 = mybir.dt.float32
TANH = mybir.ActivationFunctionType.Tanh

# blob column layout (fp16 words per partition): transposed weights only
WHH0 = 0                     # whhT: (d,k,m) -> WHH0 + (d*K+k)*H + m*P, d=0..2
WIH0 = WHH0 + D * K * H      # wihT: (d,k,m) -> WIH0 + ((d-1)*K+k)*H + m*P, d=1..2
CW = WIH0 + (D - 1) * K * H

# aux tensor layout (2 partitions, fp16): rank-2 arming operands
A_P0T = 0                    # pre0T: A_P0T + s*P + p, s = 0..S-1
A_B12 = A_P0T + S * P        # bias12T: A_B12 + (d-1)*P + p, d = 1..2
A_IND = A_B12 + (D - 1) * P  # indicator [2, KT]: ind[c, m*T+j] = (c == m)
AW = A_IND + KT

OCHUNK = 32

_cache = {}


def _patched_drain_and_barrier(self, tick_clock, wait_clock):
    """Replacement for TileContext._drain_and_barrier.

    This walrus build lowers at most ONE sync-wait per instruction; the stock
    tail drain carries one wait per active proc.  Semantically the waits only
    need to complete before the final barrier's semaphore cleanup, so spread
    them over single-wait NOPs on the sync engine after the drain.
    """
    drain_inst = self.nc.sync.drain()
    wait_clock.add_sem_waits(
        drain_inst.ins, tile.ScopedClock({None: tick_clock.global_clock})
    )
    ins = drain_inst.ins
    si = ins.sync_info
    if si is not None and len(si.on_wait) > 1:
        waits = list(si.on_wait)
        ins.sync_info = mybir.SyncInfo(on_wait=[waits[0]],
                                       on_update=list(si.on_update))
        for w in waits[1:]:
            nop = self.nc.sync.nop(nofuse=True)
            nop.ins.sync_info = mybir.SyncInfo(on_wait=[w], on_update=[])

    self.nc.all_engine_barrier()
    assert self.sems is not None
    popped = self.nc._tile_sem_poison_stack.pop()
    assert popped is self._sem_poison
    self.nc.clear_and_free_semaphores(list(self.sems.allocated().values()))
    self.nc.all_engine_barrier()


tile.TileContext._drain_and_barrier = _patched_drain_and_barrier


def _patch_ldw_opt():
    """Compile with walrus --enable-ldw-opt=true.

    Every matmul here carries a fresh stationary (the recurrence cycles 23
    weight tiles per tick), so the separate LDWEIGHTS+MATMUL pairs the
    default pipeline emits serialize the PE (~149ns/pair vs 53ns of math).
    ldw-opt lets walrus overlap/merge the weight loads.
    """
    import concourse.bass_utils as _bu
    if getattr(_bu.run_command, "_ldw_patched", False):
        return
    orig = _bu.run_command

    def run_command(cmd, *a, **kw):
        if LDW_OPT and isinstance(cmd, list):
            cmd = ["--enable-ldw-opt=true" if c == "--enable-ldw-opt=false"
                   else c for c in cmd]
        return orig(cmd, *a, **kw)

    run_command._ldw_patched = True
    _bu.run_command = run_command


LDW_OPT = True
_patch_ldw_opt()


def _build():
    nc = bass.Bass(trn_type="TRN2")

    blob = nc.dram_tensor("blob", [P, CW], F16, kind="ExternalInput")
    aux = nc.dram_tensor("aux", [K, AW], F16, kind="ExternalInput")
    # DRAM layout mirrors SBUF exactly ([p, s, kt]); host reassembles H.
    out = nc.dram_tensor("out", [P, S, KT], F16, kind="ExternalOutput")
    out_c = out[:, :, :]

    with tile.TileContext(nc) as tc:
        with (
            tc.tile_pool(name="consts", bufs=1) as consts,
            tc.tile_pool(name="u0p", bufs=4) as u0p,
            tc.tile_pool(name="u1p", bufs=4) as u1p,
            tc.tile_pool(name="ps0", bufs=2, space="PSUM") as ps0p,
            tc.tile_pool(name="ps1", bufs=2, space="PSUM") as ps1p,
            tc.tile_pool(name="ps2", bufs=2, space="PSUM") as ps2p,
            tc.tile_pool(name="psd", bufs=1, space="PSUM") as psdp,
        ):
            cb = consts.tile([P, CW], F16)
            nc.gpsimd.dma_start(out=cb, in_=blob[:, :])
            ax = consts.tile([K, AW], F16)
            nc.gpsimd.dma_start(out=ax, in_=aux[:, :])

            def whh(d, k, m):
                c = WHH0 + (d * K + k) * H + m * P
                return cb[:, c:c + P]

            def wih(d, k, m):
                c = WIH0 + ((d - 1) * K + k) * H + m * P
                return cb[:, c:c + P]

            def arm_lhsT(d, t):
                if d == 0:
                    c = A_P0T + t * P
                else:
                    c = A_B12 + (d - 1) * P
                return ax[:, c:c + P]

            ind = ax[:, A_IND:A_IND + KT]

            zeros = consts.tile([P, KT], F16)
            nc.vector.memset(zeros, 0.0)
            u2all = consts.tile([P, S, KT], F16)

            # PE absorbers + warmup: the two input DMAs land on different
            # SWDGE queues, so one warmup matmul per tensor folds each DMA
            # semaphore into PE's clock.  start=True is safe: the dummy
            # owns its full bank.
            dummy = psdp.tile([P, 512], F32, tag="init")
            nc.tensor.matmul(dummy[0:32, 0:32], lhsT=cb[0:2, 0:32],
                             rhs=cb[0:2, 0:32], start=True, stop=True)
            nc.tensor.matmul(dummy[0:32, 64:96], lhsT=ax[:, 0:32],
                             rhs=ax[:, 0:32], start=False, stop=True,
                             skip_group_check=True)

            def arm_mm(ps, d, t):
                """Rank-2 arming matmul: ps[:, m*T+j] = bias_d[m*128+p]."""
                nc.tensor.matmul(ps[:, 0:KT], lhsT=arm_lhsT(d, t), rhs=ind,
                                 start=True, stop=False, skip_group_check=True)

            def rec_mms(ps, d, u_in, u_prev):
                """ps[:, m*T:(m+1)*T] += (Wih[d] @ u_in + Whh[d] @ u_prev)."""
                per_m = (K if u_in is not None else 0) + K
                n, total = 0, per_m * K
                for m in range(K):
                    o = ps[:, m * T:(m + 1) * T]
                    if u_in is not None:
                        for k in range(K):
                            n += 1
                            nc.tensor.matmul(
                                o, lhsT=wih(d, k, m),
                                rhs=u_in[:, k * T:(k + 1) * T],
                                start=False, stop=(n == total),
                                skip_group_check=True)
                    for k in range(K):
                        n += 1
                        nc.tensor.matmul(
                            o, lhsT=whh(d, k, m),
                            rhs=u_prev[:, k * T:(k + 1) * T],
                            start=False, stop=(n == total),
                            skip_group_check=True)

            u0, u1 = {}, {}
            u0[-1] = zeros
            u1[-1] = zeros

            for t in range(S + 2):
                # ---- arm all active depths' PSUM tiles up front: one
                # contraction-2 group per tick instead of three, so the PE
                # pays the row-group reconfig bubble twice per tick, not six
                # times.
                ps0 = ps1 = ps2 = None
                if t < S:
                    ps0 = ps0p.tile([P, 512], F32, tag="ps0")
                    arm_mm(ps0, 0, t)
                if 1 <= t <= S:
                    ps1 = ps1p.tile([P, 512], F32, tag="ps1")
                    arm_mm(ps1, 1, t)
                if 2 <= t:
                    ps2 = ps2p.tile([P, 512], F32, tag="ps2")
                    arm_mm(ps2, 2, t)

                # ---- depth 0: u0[t] = tanh(pre0[:,t] + Whh0 @ u0[t-1])
                if t < S:
                    rec_mms(ps0[:, 0:KT], 0, None, u0[t - 1])
                    u = u0p.tile([P, KT], F16, tag="u0")
                    nc.scalar.activation(u, ps0[:, 0:KT], TANH)
                    u0[t] = u

                # ---- depth 1
                if 1 <= t <= S:
                    s = t - 1
                    rec_mms(ps1[:, 0:KT], 1, u0[s], u1[s - 1])
                    u = u1p.tile([P, KT], F16, tag="u1")
                    nc.scalar.activation(u, ps1[:, 0:KT], TANH)
                    u1[s] = u

                # ---- depth 2 (output depth, accumulated in u2all)
                if 2 <= t:
                    s = t - 2
                    u2_prev = zeros if s == 0 else u2all[:, s - 1, :]
                    rec_mms(ps2[:, 0:KT], 2, u1[s], u2_prev)
                    nc.scalar.activation(u2all[:, s, :], ps2[:, 0:KT], TANH)
                    if (s + 1) % OCHUNK == 0:
                        c0 = s + 1 - OCHUNK
                        nc.gpsimd.dma_start(
                            out=out_c[:, c0:c0 + OCHUNK, :],
                            in_=u2all[:, c0:c0 + OCHUNK, :])

                for dd in (u0, u1):
                    dd.pop(t - 4, None)

    _strip_same_engine_waits(nc)
    _fuse_ldweights(nc)
    return nc


def _fuse_ldweights(nc):
    """Convert split LDWEIGHTS+MATMUL pairs back to self-loading matmuls.

    The scheduler splits every 2-byte matmul into an explicit InstLdweights
    followed by the InstMatmult (ldweights=False).  On hardware the pair
    executes serially (~149ns vs 53ns of math for a 128x128 fp16 tile).  The
    split matmul still carries both operands, so dropping the InstLdweights
    and restoring ldweights=None yields the fp32-style self-loading form.
    """
    for fn in nc.m.functions:
        for blk in fn.blocks:
            insts = blk.instructions
            out, pending = [], None
            for inst in insts:
                if type(inst).__name__ == "InstLdweights":
                    assert pending is None
                    pending = inst
                    continue
                if pending is not None:
                    assert type(inst).__name__ == "InstMatmult", inst
                    inst.ldweights = None
                    psi, si = pending.sync_info, inst.sync_info
                    waits = list(psi.on_wait if psi else []) + \
                        list(si.on_wait if si else [])
                    upds = list(psi.on_update if psi else []) + \
                        list(si.on_update if si else [])
                    assert len(waits) <= 1, waits
                    if waits or upds:
                        inst.sync_info = mybir.SyncInfo(
                            on_wait=waits, on_update=upds)
                    pending = None
                out.append(inst)
            assert pending is None
            if len(out) != len(insts):
                blk.instructions = out


_ENG_SEM_PREFIX = {
    mybir.EngineType.Activation: "Activation",
    mybir.EngineType.PE: "PE",
    mybir.EngineType.DVE: "DVE",
    mybir.EngineType.Pool: "Pool",
    mybir.EngineType.SP: "SP",
}


def _strip_same_engine_waits(nc):
    """Drop sem waits an instruction holds on its OWN engine's stream sem.

    Tile emits pool-reuse WAW/WAR hazards as explicit sem waits even when
    producer and consumer share an engine; same-engine execution is in-order
    so such waits are provably satisfied at issue.  Removing them keeps every
    instruction at <=1 sync-wait (a hard limit of this walrus build).
    """
    for fn in nc.m.functions:
        for blk in fn.blocks:
            for inst in blk.instructions:
                si = inst.sync_info
                if si is None or not si.on_wait:
                    continue
                pfx = _ENG_SEM_PREFIX.get(inst.engine)
                if pfx is None:
                    continue
                keep = [w for w in si.on_wait
                        if w.ant_name.rsplit("_", 1)[0] != pfx]
                if len(keep) != len(si.on_wait):
                    inst.sync_info = mybir.SyncInfo(
                        on_wait=keep, on_update=list(si.on_update))
                assert len(keep) <= 1, (
                    f"{inst.name}: {len(keep)} foreign waits remain: "
                    f"{[w.ant_name for w in keep]}")


def _blob(whhT, wihT12):
    """Pack per-core transposed weights into the [P, CW] fp16 blob."""
    b = np.empty((P, CW), np.float16)
    b[:, WHH0:WHH0 + D * K * H] = (
        whhT.reshape(D, K, P, H).transpose(2, 0, 1, 3).reshape(P, D * K * H))
    b[:, WIH0:WIH0 + (D - 1) * K * H] = (
        wihT12.reshape(D - 1, K, P, H).transpose(2, 0, 1, 3)
        .reshape(P, (D - 1) * K * H))
    return b


def _aux(W0, seed, b0, bias12):
    """Rank-2 arming operands on partitions 0..1: pre0T, bias12T, indicator."""
    a = np.zeros((K, AW), np.float16)
    # pre0[h, s] = (W0 @ seed.T + b0)[h]; a[c, s*P+p] = pre0[c*128+p, s]
    pre0 = W0 @ seed.T + b0[:, None]
    a[:, A_P0T:A_P0T + S * P] = (
        pre0.reshape(K, P, S).transpose(0, 2, 1).reshape(K, S * P))
    # a[c, (d-1)*P+p] = bias12[d-1, c*128+p]
    a[:, A_B12:A_B12 + (D - 1) * P] = (
        bias12.reshape(D - 1, K, P).transpose(1, 0, 2).reshape(K, (D - 1) * P))
    for c in range(K):
        a[c, A_IND + c * T:A_IND + (c + 1) * T] = 1.0
    return a


def kernel(src, trg, Wx_ih, Wx_hh, bx_ih, bx_hh, Wy_ih, Wy_hh, by_ih, by_hh):
    if "nc" not in _cache:
        _cache["nc"] = _build()
    nc = _cache["nc"]

    def tr(w):  # [D,H,H] -> W[d].T contiguous, fp16
        return np.ascontiguousarray(
            np.swapaxes(np.asarray(w, np.float32), 1, 2)).astype(np.float16)

    src = np.asarray(src, np.float32)
    trg = np.asarray(trg, np.float32)
    blob_x = _blob(tr(Wx_hh), tr(Wx_ih)[1:])
    blob_y = _blob(tr(Wy_hh), tr(Wy_ih)[1:])
    bx = np.asarray(bx_ih, np.float32) + np.asarray(bx_hh, np.float32)
    by = np.asarray(by_ih, np.float32) + np.asarray(by_hh, np.float32)
    Wx0 = np.asarray(Wx_ih, np.float32)[0]
    Wy0 = np.asarray(Wy_ih, np.float32)[0]

    in_maps = []
    for b in range(B):  # cores 0-3: x chains
        in_maps.append({"blob": blob_x,
                        "aux": _aux(Wx0, src[b], bx[0], bx[1:])})
    for b in range(B):  # cores 4-7: y chains
        in_maps.append({"blob": blob_y,
                        "aux": _aux(Wy0, trg[b], by[0], by[1:])})

    _cache["last_in_maps"] = in_maps
    globals()["_last_in_maps"] = in_maps
    res = run_bass_kernel_spmd(nc, in_maps, list(range(8)))

    out = np.empty((B, S, T, 2, H), np.float32)
    ii = np.arange(S)[:, None]
    jj = np.arange(T)[None, :]
    idx = (jj - ii) % T  # hx[i,j] = u_i[(j-i)%T]
    for b in range(B):
        # raw core output [p, s, k*T+v] -> [s, H=k*128+p, v]
        arr = np.asarray(res.results[b]["out"], np.float32)
        arr = arr.reshape(P, S, K, T).transpose(1, 2, 0, 3).reshape(S, H, T)
        hx = np.take_along_axis(arr, idx[:, None, :], axis=2)  # [s, H, j]
        out[b, :, :, 0, :] = hx.transpose(0, 2, 1)
        arr = np.asarray(res.results[B + b]["out"], np.float32)
        arr = arr.reshape(P, S, K, T).transpose(1, 2, 0, 3).reshape(S, H, T)
        out[b, :, :, 1, :] = arr.transpose(2, 0, 1)  # [j, H, i] -> [i, j, H]
    return out


# revision 18
# speedup vs baseline: 2.9130x; 1.1476x over previous
"""GridRNN Trainium2 kernel (fp16 matmul path).

Problem: 2-D grid RNN, B=4, S=T=128, H=256, D=3 depths.
  hx[d][b,i,j] = tanh(xin @ Wx_ih[d].T + bx_ih[d] + hx[d][b,i-1,(j-1)%T] @ Wx_hh[d].T + bx_hh[d])
  hy[d][b,i,j] = tanh(yin @ Wy_ih[d].T + by_ih[d] + hy[d][b,i,j-1]     @ Wy_hh[d].T + by_hh[d])
  (xin/yin = src/trg broadcast at d=0, previous depth's hx/hy for d>0)
  out = stack([hx[D-1], hy[D-1]], axis=-2)   # [B,S,T,2,H]

8 cores = 4 batches x 2 chains (the x-chain's diagonal dependence is removed
by shearing u_i[c] = hx[i,(i+c)%T], making both chains plain carries).  One
SPMD program; only per-core input data differs.  Host unshears x, transposes y.

Perf design (vs the fp32 baseline at ~616us):
- All matmuls fp16: 1 PE cycle/row instead of fp32's 4.
- Depth-0's input term (W_ih0 @ seed + b) is precomputed on the HOST (fp32)
  and shipped transposed as per-step columns; no on-device seed transpose.
- Biases never ride the activation (the bias AP must be free-size-1, which
  would force 2 acts per depth): instead each depth's PSUM tile is pre-armed
  each tick by a rank-2 PE matmul
      ps[p, m*T+j] = sum_c lhsT[c,p] * ind[c, m*T+j],  ind[c,.] = block c
  with lhsT = the depth's bias pair (pre0[:, s] for depth 0) on partitions
  0..1.  Rec matmuls then accumulate with start=False and ONE fused
  [128, K*T] tanh per depth per tick keeps ScalarE at 3 instrs/tick.
- Tile's vector clocks are NOT transitive across engines and this walrus
  build lowers at most ONE sync-wait per instruction, so every instruction
  may depend on at most ONE foreign engine: arming on the PE makes each
  PSUM tile PE-write-only (WAW free), acts depend only on PE, arming
  matmuls' WAR-vs-act deps are covered by the earlier same-tick rec-matmul
  waits, and the one-time DMA semaphore is absorbed into PE's clock by a
  warmup matmul.  Each PSUM tile owns a full 2KB bank so start=True's lazy
  zeroing (which marks the whole 2KB "zero region") cannot poison others.
"""

import numpy as np

import concourse.bass as bass
import concourse.tile as tile
from concourse import mybir
from concourse.bass_utils import run_bass_kernel_spmd

B, S, T, H, D = 4, 128, 128, 256, 3
P = 128          # partitions
K = H // P       # 2 k-tiles of H on partitions
KT = K * T       # 256: one depth's full output row block
F16 = mybir.dt.float16
F32 = mybir.dt.float32
TANH = mybir.ActivationFunctionType.Tanh

# blob column layout (fp16 words per partition): transposed weights only
WHH0 = 0                     # whhT: (d,k,m) -> WHH0 + (d*K+k)*H + m*P, d=0..2
WIH0 = WHH0 + D * K * H      # wihT: (d,k,m) -> WIH0 + ((d-1)*K+k)*H + m*P, d=1..2
CW = WIH0 + (D - 1) * K * H

# aux tensor layout (fp16): arming operands.  Only rows 0..1 carry data;
# rows 2..127 are host-packed ZEROS so the arming matmuls can run at
# contraction 128 (same PE row-group as the recurrence matmuls — avoids the
# expensive row-group reconfig) with the padding contributing exact zeros.
A_B12 = 0                    # bias12T: A_B12 + (d-1)*P + p, d = 1..2
A_IND = A_B12 + (D - 1) * P  # indicator: ind[c, m*T+j] = (c == m), c < K
A_P0T = A_IND + KT           # pre0T: A_P0T + s*P + p, s = 0..S-1
AW = A_P0T + S * P
A_SPLIT = A_P0T + 16 * P     # first DMA covers bias/ind/pre0T[:16]

OCHUNK = 32

_cache = {}


def _patched_drain_and_barrier(self, tick_clock, wait_clock):
    """Replacement for TileContext._drain_and_barrier.

    This walrus build lowers at most ONE sync-wait per instruction; the stock
    tail drain carries one wait per active proc.  Semantically the waits only
    need to complete before the final barrier's semaphore cleanup, so spread
    them over single-wait NOPs on the sync engine after the drain.
    """
    drain_inst = self.nc.sync.drain()
    wait_clock.add_sem_waits(
        drain_inst.ins, tile.ScopedClock({None: tick_clock.global_clock})
    )
    ins = drain_inst.ins
    si = ins.sync_info
    if si is not None and len(si.on_wait) > 1:
        waits = list(si.on_wait)
        ins.sync_info = mybir.SyncInfo(on_wait=[waits[0]],
                                       on_update=list(si.on_update))
        for w in waits[1:]:
            nop = self.nc.sync.nop(nofuse=True)
            nop.ins.sync_info = mybir.SyncInfo(on_wait=[w], on_update=[])

    self.nc.all_engine_barrier()
    assert self.sems is not None
    popped = self.nc._tile_sem_poison_stack.pop()
    assert popped is self._sem_poison
    self.nc.clear_and_free_semaphores(list(self.sems.allocated().values()))
    self.nc.all_engine_barrier()


tile.TileContext._drain_and_barrier = _patched_drain_and_barrier


def _patch_ldw_opt():
    """Compile with walrus --enable-ldw-opt=true.

    Every matmul here carries a fresh stationary (the recurrence cycles 23
    weight tiles per tick), so the separate LDWEIGHTS+MATMUL pairs the
    default pipeline emits serialize the PE (~149ns/pair vs 53ns of math).
    ldw-opt lets walrus overlap/merge the weight loads.
    """
    import concourse.bass_utils as _bu
    if getattr(_bu.run_command, "_ldw_patched", False):
        return
    orig = _bu.run_command

    def run_command(cmd, *a, **kw):
        if LDW_OPT and isinstance(cmd, list):
            cmd = ["--enable-ldw-opt=true" if c == "--enable-ldw-opt=false"
                   else c for c in cmd]
        return orig(cmd, *a, **kw)

    run_command._ldw_patched = True
    _bu.run_command = run_command


LDW_OPT = True
_patch_ldw_opt()


def _build():
    nc = bass.Bass(trn_type="TRN2")

    blob = nc.dram_tensor("blob", [P, CW], F16, kind="ExternalInput")
    aux = nc.dram_tensor("aux", [P, AW], F16, kind="ExternalInput")
    # DRAM layout mirrors SBUF exactly ([p, s, kt]); host reassembles H.
    out = nc.dram_tensor("out", [P, S, KT], F16, kind="ExternalOutput")
    out_c = out[:, :, :]

    with tile.TileContext(nc) as tc:
        with (
            tc.tile_pool(name="consts", bufs=1) as consts,
            tc.tile_pool(name="u0p", bufs=4) as u0p,
            tc.tile_pool(name="u1p", bufs=4) as u1p,
            tc.tile_pool(name="ps0", bufs=2, space="PSUM") as ps0p,
            tc.tile_pool(name="ps1", bufs=2, space="PSUM") as ps1p,
            tc.tile_pool(name="ps2", bufs=2, space="PSUM") as ps2p,
            tc.tile_pool(name="psd", bufs=1, space="PSUM") as psdp,
        ):
            cb = consts.tile([P, CW], F16)
            nc.gpsimd.dma_start(out=cb, in_=blob[:, :])
            # aux in two chunks so tick 0 isn't gated on all of pre0T.
            ax = consts.tile([P, AW], F16)
            nc.gpsimd.dma_start(out=ax[:, 0:A_SPLIT], in_=aux[:, 0:A_SPLIT])
            nc.gpsimd.dma_start(out=ax[:, A_SPLIT:AW], in_=aux[:, A_SPLIT:AW])

            def whh(d, k, m):
                c = WHH0 + (d * K + k) * H + m * P
                return cb[:, c:c + P]

            def wih(d, k, m):
                c = WIH0 + ((d - 1) * K + k) * H + m * P
                return cb[:, c:c + P]

            def arm_lhsT(d, t):
                if d == 0:
                    c = A_P0T + t * P
                else:
                    c = A_B12 + (d - 1) * P
                return ax[:, c:c + P]

            ind = ax[:, A_IND:A_IND + KT]

            zeros = consts.tile([P, KT], F16)
            nc.vector.memset(zeros, 0.0)
            u2all = consts.tile([P, S, KT], F16)

            # PE absorbers + warmup: the two input DMAs land on different
            # SWDGE queues, so one warmup matmul per tensor folds each DMA
            # semaphore into PE's clock.  start=True is safe: the dummy
            # owns its full bank.
            dummy = psdp.tile([P, 512], F32, tag="init")
            nc.tensor.matmul(dummy[0:32, 0:32], lhsT=cb[0:2, 0:32],
                             rhs=cb[0:2, 0:32], start=True, stop=True)
            nc.tensor.matmul(dummy[0:32, 64:96], lhsT=ax[:, 0:32],
                             rhs=ax[:, 0:32], start=False, stop=True,
                             skip_group_check=True)

            def arm_mm(ps, d, t):
                """Arming matmul: ps[:, m*T+j] = bias_d[m*128+p].

                Contraction 128 (rows 2..127 are zeros on both operands) so
                it shares the recurrence matmuls' PE row-group config.
                """
                nc.tensor.matmul(ps[:, 0:KT], lhsT=arm_lhsT(d, t), rhs=ind,
                                 start=True, stop=False, skip_group_check=True)

            def rec_mms(ps, d, u_in, u_prev):
                """ps[:, m*T:(m+1)*T] += (Wih[d] @ u_in + Whh[d] @ u_prev)."""
                per_m = (K if u_in is not None else 0) + K
                n, total = 0, per_m * K
                for m in range(K):
                    o = ps[:, m * T:(m + 1) * T]
                    if u_in is not None:
                        for k in range(K):
                            n += 1
                            nc.tensor.matmul(
                                o, lhsT=wih(d, k, m),
                                rhs=u_in[:, k * T:(k + 1) * T],
                                start=False, stop=(n == total),
                                skip_group_check=True)
                    for k in range(K):
                        n += 1
                        nc.tensor.matmul(
                            o, lhsT=whh(d, k, m),
                            rhs=u_prev[:, k * T:(k + 1) * T],
                            start=False, stop=(n == total),
                            skip_group_check=True)

            u0, u1 = {}, {}
            u0[-1] = zeros
            u1[-1] = zeros

            for t in range(S + 2):
                # ---- arm all active depths' PSUM tiles up front: one
                # contraction-2 group per tick instead of three, so the PE
                # pays the row-group reconfig bubble twice per tick, not six
                # times.
                ps0 = ps1 = ps2 = None
                if t < S:
                    ps0 = ps0p.tile([P, 512], F32, tag="ps0")
                    arm_mm(ps0, 0, t)
                if 1 <= t <= S:
                    ps1 = ps1p.tile([P, 512], F32, tag="ps1")
                    arm_mm(ps1, 1, t)
                if 2 <= t:
                    ps2 = ps2p.tile([P, 512], F32, tag="ps2")
                    arm_mm(ps2, 2, t)

                # ---- depth 0: u0[t] = tanh(pre0[:,t] + Whh0 @ u0[t-1])
                if t < S:
                    rec_mms(ps0[:, 0:KT], 0, None, u0[t - 1])
                    u = u0p.tile([P, KT], F16, tag="u0")
                    nc.scalar.activation(u, ps0[:, 0:KT], TANH)
                    u0[t] = u

                # ---- depth 1
                if 1 <= t <= S:
                    s = t - 1
                    rec_mms(ps1[:, 0:KT], 1, u0[s], u1[s - 1])
                    u = u1p.tile([P, KT], F16, tag="u1")
                    nc.scalar.activation(u, ps1[:, 0:KT], TANH)
                    u1[s] = u

                # ---- depth 2 (output depth, accumulated in u2all)
                if 2 <= t:
                    s = t - 2
                    u2_prev = zeros if s == 0 else u2all[:, s - 1, :]
                    rec_mms(ps2[:, 0:KT], 2, u1[s], u2_prev)
                    nc.scalar.activation(u2all[:, s, :], ps2[:, 0:KT], TANH)
                    if (s + 1) % OCHUNK == 0:
                        c0 = s + 1 - OCHUNK
                        nc.gpsimd.dma_start(
                            out=out_c[:, c0:c0 + OCHUNK, :],
                            in_=u2all[:, c0:c0 + OCHUNK, :])

                for dd in (u0, u1):
                    dd.pop(t - 4, None)

    _strip_same_engine_waits(nc)
    _fuse_ldweights(nc)
    return nc


def _fuse_ldweights(nc):
    """Convert split LDWEIGHTS+MATMUL pairs back to self-loading matmuls.

    The scheduler splits every 2-byte matmul into an explicit InstLdweights
    followed by the InstMatmult (ldweights=False).  On hardware the pair
    executes serially (~149ns vs 53ns of math for a 128x128 fp16 tile).  The
    split matmul still carries both operands, so dropping the InstLdweights
    and restoring ldweights=None yields the fp32-style self-loading form.
    """
    for fn in nc.m.functions:
        for blk in fn.blocks:
            insts = blk.instructions
            out, pending = [], None
            for inst in insts:
                if type(inst).__name__ == "InstLdweights":
                    assert pending is None
                    pending = inst
                    continue
                if pending is not None:
                    assert type(inst).__name__ == "InstMatmult", inst
                    inst.ldweights = None
                    psi, si = pending.sync_info, inst.sync_info
                    waits = list(psi.on_wait if psi else []) + \
                        list(si.on_wait if si else [])
                    upds = list(psi.on_update if psi else []) + \
                        list(si.on_update if si else [])
                    assert len(waits) <= 1, waits
                    if waits or upds:
                        inst.sync_info = mybir.SyncInfo(
                            on_wait=waits, on_update=upds)
                    pending = None
                out.append(inst)
            assert pending is None
            if len(out) != len(insts):
                blk.instructions = out


_ENG_SEM_PREFIX = {
    mybir.EngineType.Activation: "Activation",
    mybir.EngineType.PE: "PE",
    mybir.EngineType.DVE: "DVE",
    mybir.EngineType.Pool: "Pool",
    mybir.EngineType.SP: "SP",
}


def _strip_same_engine_waits(nc):
    """Drop sem waits an instruction holds on its OWN engine's stream sem.

    Tile emits pool-reuse WAW/WAR hazards as explicit sem waits even when
    producer and consumer share an engine; same-engine execution is in-order
    so such waits are provably satisfied at issue.  Removing them keeps every
    instruction at <=1 sync-wait (a hard limit of this walrus build).
    """
    for fn in nc.m.functions:
        for blk in fn.blocks:
            for inst in blk.instructions:
                si = inst.sync_info
                if si is None or not si.on_wait:
                    continue
                pfx = _ENG_SEM_PREFIX.get(inst.engine)
                if pfx is None:
                    continue
                keep = [w for w in si.on_wait
                        if w.ant_name.rsplit("_", 1)[0] != pfx]
                if len(keep) != len(si.on_wait):
                    inst.sync_info = mybir.SyncInfo(
                        on_wait=keep, on_update=list(si.on_update))
                assert len(keep) <= 1, (
                    f"{inst.name}: {len(keep)} foreign waits remain: "
                    f"{[w.ant_name for w in keep]}")


def _blob(whhT, wihT12):
    """Pack per-core transposed weights into the [P, CW] fp16 blob."""
    b = np.empty((P, CW), np.float16)
    b[:, WHH0:WHH0 + D * K * H] = (
        whhT.reshape(D, K, P, H).transpose(2, 0, 1, 3).reshape(P, D * K * H))
    b[:, WIH0:WIH0 + (D - 1) * K * H] = (
        wihT12.reshape(D - 1, K, P, H).transpose(2, 0, 1, 3)
        .reshape(P, (D - 1) * K * H))
    return b


def _aux(W0, seed, b0, bias12):
    """Arming operands on rows 0..1 of a zero [P, AW] sheet (the zero rows
    let the arming matmuls run at contraction 128)."""
    a = np.zeros((P, AW), np.float16)
    # pre0[h, s] = (W0 @ seed.T + b0)[h]; a[c, s*P+p] = pre0[c*128+p, s]
    pre0 = W0 @ seed.T + b0[:, None]
    a[0:K, A_P0T:A_P0T + S * P] = (
        pre0.reshape(K, P, S).transpose(0, 2, 1).reshape(K, S * P))
    # a[c, (d-1)*P+p] = bias12[d-1, c*128+p]
    a[0:K, A_B12:A_B12 + (D - 1) * P] = (
        bias12.reshape(D - 1, K, P).transpose(1, 0, 2).reshape(K, (D - 1) * P))
    for c in range(K):
        a[c, A_IND + c * T:A_IND + (c + 1) * T] = 1.0
    return a


def kernel(src, trg, Wx_ih, Wx_hh, bx_ih, bx_hh, Wy_ih, Wy_hh, by_ih, by_hh):
    if "nc" not in _cache:
        _cache["nc"] = _build()
    nc = _cache["nc"]

    def tr(w):  # [D,H,H] -> W[d].T contiguous, fp16
        return np.ascontiguousarray(
            np.swapaxes(np.asarray(w, np.float32), 1, 2)).astype(np.float16)

    src = np.asarray(src, np.float32)
    trg = np.asarray(trg, np.float32)
    blob_x = _blob(tr(Wx_hh), tr(Wx_ih)[1:])
    blob_y = _blob(tr(Wy_hh), tr(Wy_ih)[1:])
    bx = np.asarray(bx_ih, np.float32) + np.asarray(bx_hh, np.float32)
    by = np.asarray(by_ih, np.float32) + np.asarray(by_hh, np.float32)
    Wx0 = np.asarray(Wx_ih, np.float32)[0]
    Wy0 = np.asarray(Wy_ih, np.float32)[0]

    in_maps = []
    for b in range(B):  # cores 0-3: x chains
        in_maps.append({"blob": blob_x,
                        "aux": _aux(Wx0, src[b], bx[0], bx[1:])})
    for b in range(B):  # cores 4-7: y chains
        in_maps.append({"blob": blob_y,
                        "aux": _aux(Wy0, trg[b], by[0], by[1:])})

    _cache["last_in_maps"] = in_maps
    globals()["_last_in_maps"] = in_maps
    res = run_bass_kernel_spmd(nc, in_maps, list(range(8)))

    out = np.empty((B, S, T, 2, H), np.float32)
    ii = np.arange(S)[:, None]
    jj = np.arange(T)[None, :]
    idx = (jj - ii) % T  # hx[i,j] = u_i[(j-i)%T]
    for b in range(B):
        # raw core output [p, s, k*T+v] -> [s, H=k*128+p, v]
        arr = np.asarray(res.results[b]["out"], np.float32)
        arr = arr.reshape(P, S, K, T).transpose(1, 2, 0, 3).reshape(S, H, T)
        hx = np.take_along_axis(arr, idx[:, None, :], axis=2)  # [s, H, j]
        out[b, :, :, 0, :] = hx.transpose(0, 2, 1)
        arr = np.asarray(res.results[B + b]["out"], np.float32)
        arr = arr.reshape(P, S, K, T).transpose(1, 2, 0, 3).reshape(S, H, T)
        out[b, :, :, 1, :] = arr.transpose(2, 0, 1)  # [j, H, i] -> [i, j, H]
    return out


# revision 22
# speedup vs baseline: 2.9916x; 1.0270x over previous
"""GridRNN Trainium2 kernel (fp16 matmul path).

Problem: 2-D grid RNN, B=4, S=T=128, H=256, D=3 depths.
  hx[d][b,i,j] = tanh(xin @ Wx_ih[d].T + bx_ih[d] + hx[d][b,i-1,(j-1)%T] @ Wx_hh[d].T + bx_hh[d])
  hy[d][b,i,j] = tanh(yin @ Wy_ih[d].T + by_ih[d] + hy[d][b,i,j-1]     @ Wy_hh[d].T + by_hh[d])
  (xin/yin = src/trg broadcast at d=0, previous depth's hx/hy for d>0)
  out = stack([hx[D-1], hy[D-1]], axis=-2)   # [B,S,T,2,H]

8 cores = 4 batches x 2 chains (the x-chain's diagonal dependence is removed
by shearing u_i[c] = hx[i,(i+c)%T], making both chains plain carries).  One
SPMD program; only per-core input data differs.  Host unshears x, transposes y.

Perf design (vs the fp32 baseline at ~616us):
- All matmuls fp16: 1 PE cycle/row instead of fp32's 4.
- Depth-0's input term (W_ih0 @ seed + b) is precomputed on the HOST (fp32)
  and shipped transposed as per-step columns; no on-device seed transpose.
- Biases never ride the activation (the bias AP must be free-size-1, which
  would force 2 acts per depth): instead each depth's PSUM tile is pre-armed
  each tick by a rank-2 PE matmul
      ps[p, m*T+j] = sum_c lhsT[c,p] * ind[c, m*T+j],  ind[c,.] = block c
  with lhsT = the depth's bias pair (pre0[:, s] for depth 0) on partitions
  0..1.  Rec matmuls then accumulate with start=False and ONE fused
  [128, K*T] tanh per depth per tick keeps ScalarE at 3 instrs/tick.
- Tile's vector clocks are NOT transitive across engines and this walrus
  build lowers at most ONE sync-wait per instruction, so every instruction
  may depend on at most ONE foreign engine: arming on the PE makes each
  PSUM tile PE-write-only (WAW free), acts depend only on PE, arming
  matmuls' WAR-vs-act deps are covered by the earlier same-tick rec-matmul
  waits, and the one-time DMA semaphore is absorbed into PE's clock by a
  warmup matmul.  Each PSUM tile owns a full 2KB bank so start=True's lazy
  zeroing (which marks the whole 2KB "zero region") cannot poison others.
"""

import numpy as np

import concourse.bass as bass
import concourse.tile as tile
from concourse import mybir
from concourse.bass_utils import run_bass_kernel_spmd

B, S, T, H, D = 4, 128, 128, 256, 3
P = 128          # partitions
K = H // P       # 2 k-tiles of H on partitions
KT = K * T       # 256: one depth's full output row block
F16 = mybir.dt.float16
F32 = mybir.dt.float32
TANH = mybir.ActivationFunctionType.Tanh

# blob column layout (fp16 words per partition): transposed weights only
WHH0 = 0                     # whhT: (d,k,m) -> WHH0 + (d*K+k)*H + m*P, d=0..2
WIH0 = WHH0 + D * K * H      # wihT: (d,k,m) -> WIH0 + ((d-1)*K+k)*H + m*P, d=1..2
CW = WIH0 + (D - 1) * K * H

# aux tensor layout (fp16): arming operands.  Only rows 0..1 carry data;
# rows 2..127 are host-packed ZEROS so the arming matmuls can run at
# contraction 128 (same PE row-group as the recurrence matmuls — avoids the
# expensive row-group reconfig) with the padding contributing exact zeros.
A_B12 = 0                    # bias12T: A_B12 + (d-1)*P + p, d = 1..2
A_IND = A_B12 + (D - 1) * P  # indicator: ind[c, m*T+j] = (c == m), c < K
A_P0T = A_IND + KT           # pre0T: A_P0T + s*P + p, s = 0..S-1
AW = A_P0T + S * P
A_SPLIT = A_P0T + 16 * P     # first DMA covers bias/ind/pre0T[:16]

# output DMA chunk boundaries: big early chunks overlap compute, tapered
# tail chunks keep the final post-compute transfer tiny
CHUNK_ENDS = [48, 96, 120, 126, 128]

_cache = {}


def _patched_drain_and_barrier(self, tick_clock, wait_clock):
    """Replacement for TileContext._drain_and_barrier.

    This walrus build lowers at most ONE sync-wait per instruction; the stock
    tail drain carries one wait per active proc.  Semantically the waits only
    need to complete before the final barrier's semaphore cleanup, so spread
    them over single-wait NOPs on the sync engine after the drain.
    """
    drain_inst = self.nc.sync.drain()
    wait_clock.add_sem_waits(
        drain_inst.ins, tile.ScopedClock({None: tick_clock.global_clock})
    )
    ins = drain_inst.ins
    si = ins.sync_info
    if si is not None and len(si.on_wait) > 1:
        waits = list(si.on_wait)
        ins.sync_info = mybir.SyncInfo(on_wait=[waits[0]],
                                       on_update=list(si.on_update))
        for w in waits[1:]:
            nop = self.nc.sync.nop(nofuse=True)
            nop.ins.sync_info = mybir.SyncInfo(on_wait=[w], on_update=[])

    self.nc.all_engine_barrier()
    assert self.sems is not None
    popped = self.nc._tile_sem_poison_stack.pop()
    assert popped is self._sem_poison
    self.nc.clear_and_free_semaphores(list(self.sems.allocated().values()))
    self.nc.all_engine_barrier()


tile.TileContext._drain_and_barrier = _patched_drain_and_barrier


def _patch_ldw_opt():
    """Compile with walrus --enable-ldw-opt=true.

    Every matmul here carries a fresh stationary (the recurrence cycles 23
    weight tiles per tick), so the separate LDWEIGHTS+MATMUL pairs the
    default pipeline emits serialize the PE (~149ns/pair vs 53ns of math).
    ldw-opt lets walrus overlap/merge the weight loads.
    """
    import concourse.bass_utils as _bu
    if getattr(_bu.run_command, "_ldw_patched", False):
        return
    orig = _bu.run_command

    def run_command(cmd, *a, **kw):
        if LDW_OPT and isinstance(cmd, list):
            cmd = ["--enable-ldw-opt=true" if c == "--enable-ldw-opt=false"
                   else c for c in cmd]
        return orig(cmd, *a, **kw)

    run_command._ldw_patched = True
    _bu.run_command = run_command


LDW_OPT = True
_patch_ldw_opt()


def _build():
    nc = bass.Bass(trn_type="TRN2")

    blob = nc.dram_tensor("blob", [P, CW], F16, kind="ExternalInput")
    aux = nc.dram_tensor("aux", [P, AW], F16, kind="ExternalInput")
    # DRAM layout mirrors SBUF exactly ([p, s, kt]); host reassembles H.
    out = nc.dram_tensor("out", [P, S, KT], F16, kind="ExternalOutput")
    out_c = out[:, :, :]

    with tile.TileContext(nc) as tc:
        with (
            tc.tile_pool(name="consts", bufs=1) as consts,
            tc.tile_pool(name="u0p", bufs=4) as u0p,
            tc.tile_pool(name="u1p", bufs=4) as u1p,
            tc.tile_pool(name="ps0", bufs=2, space="PSUM") as ps0p,
            tc.tile_pool(name="ps1", bufs=2, space="PSUM") as ps1p,
            tc.tile_pool(name="ps2", bufs=2, space="PSUM") as ps2p,
            tc.tile_pool(name="psd", bufs=1, space="PSUM") as psdp,
        ):
            # Input DMAs issue from the idle SP engine: the gpsimd queue is
            # busy with library loads for the first ~5us and would delay the
            # transfers.  aux ships in two chunks so tick 0 isn't gated on
            # all of pre0T.
            cb = consts.tile([P, CW], F16)
            nc.sync.dma_start(out=cb, in_=blob[:, :])
            ax = consts.tile([P, AW], F16)
            nc.sync.dma_start(out=ax[:, 0:A_SPLIT], in_=aux[:, 0:A_SPLIT])
            nc.sync.dma_start(out=ax[:, A_SPLIT:AW], in_=aux[:, A_SPLIT:AW])

            def whh(d, k, m):
                c = WHH0 + (d * K + k) * H + m * P
                return cb[:, c:c + P]

            def wih(d, k, m):
                c = WIH0 + ((d - 1) * K + k) * H + m * P
                return cb[:, c:c + P]

            def arm_lhsT(d, t):
                if d == 0:
                    c = A_P0T + t * P
                else:
                    c = A_B12 + (d - 1) * P
                return ax[:, c:c + P]

            ind = ax[:, A_IND:A_IND + KT]

            zeros = consts.tile([P, KT], F16)
            nc.vector.memset(zeros, 0.0)
            u2all = consts.tile([P, S, KT], F16)

            # PE absorbers + warmup: the two input DMAs land on different
            # SWDGE queues, so one warmup matmul per tensor folds each DMA
            # semaphore into PE's clock.  start=True is safe: the dummy
            # owns its full bank.
            dummy = psdp.tile([P, 512], F32, tag="init")
            nc.tensor.matmul(dummy[0:32, 0:32], lhsT=cb[0:2, 0:32],
                             rhs=cb[0:2, 0:32], start=True, stop=True)
            nc.tensor.matmul(dummy[0:32, 64:96], lhsT=ax[:, 0:32],
                             rhs=ax[:, 0:32], start=False, stop=True,
                             skip_group_check=True)

            def arm_mm(ps, d, t):
                """Arming matmul: ps[:, m*T+j] = bias_d[m*128+p].

                Contraction 128 (rows 2..127 are zeros on both operands) so
                it shares the recurrence matmuls' PE row-group config.
                """
                nc.tensor.matmul(ps[:, 0:KT], lhsT=arm_lhsT(d, t), rhs=ind,
                                 start=True, stop=False, skip_group_check=True)

            def rec_mms(ps, d, u_in, u_prev):
                """ps[:, m*T:(m+1)*T] += (Wih[d] @ u_in + Whh[d] @ u_prev)."""
                per_m = (K if u_in is not None else 0) + K
                n, total = 0, per_m * K
                for m in range(K):
                    o = ps[:, m * T:(m + 1) * T]
                    if u_in is not None:
                        for k in range(K):
                            n += 1
                            nc.tensor.matmul(
                                o, lhsT=wih(d, k, m),
                                rhs=u_in[:, k * T:(k + 1) * T],
                                start=False, stop=(n == total),
                                skip_group_check=True)
                    for k in range(K):
                        n += 1
                        nc.tensor.matmul(
                            o, lhsT=whh(d, k, m),
                            rhs=u_prev[:, k * T:(k + 1) * T],
                            start=False, stop=(n == total),
                            skip_group_check=True)

            u0, u1 = {}, {}
            u0[-1] = zeros
            u1[-1] = zeros

            for t in range(S + 2):
                # ---- arm all active depths' PSUM tiles up front: one
                # contraction-2 group per tick instead of three, so the PE
                # pays the row-group reconfig bubble twice per tick, not six
                # times.
                ps0 = ps1 = ps2 = None
                if t < S:
                    ps0 = ps0p.tile([P, 512], F32, tag="ps0")
                    arm_mm(ps0, 0, t)
                if 1 <= t <= S:
                    ps1 = ps1p.tile([P, 512], F32, tag="ps1")
                    arm_mm(ps1, 1, t)
                if 2 <= t:
                    ps2 = ps2p.tile([P, 512], F32, tag="ps2")
                    arm_mm(ps2, 2, t)

                # ---- depth 0: u0[t] = tanh(pre0[:,t] + Whh0 @ u0[t-1])
                if t < S:
                    rec_mms(ps0[:, 0:KT], 0, None, u0[t - 1])
                    u = u0p.tile([P, KT], F16, tag="u0")
                    nc.scalar.activation(u, ps0[:, 0:KT], TANH)
                    u0[t] = u

                # ---- depth 1
                if 1 <= t <= S:
                    s = t - 1
                    rec_mms(ps1[:, 0:KT], 1, u0[s], u1[s - 1])
                    u = u1p.tile([P, KT], F16, tag="u1")
                    nc.scalar.activation(u, ps1[:, 0:KT], TANH)
                    u1[s] = u

                # ---- depth 2 (output depth, accumulated in u2all)
                if 2 <= t:
                    s = t - 2
                    u2_prev = zeros if s == 0 else u2all[:, s - 1, :]
                    rec_mms(ps2[:, 0:KT], 2, u1[s], u2_prev)
                    nc.scalar.activation(u2all[:, s, :], ps2[:, 0:KT], TANH)
                    if (s + 1) in CHUNK_ENDS:
                        c0 = CHUNK_ENDS[CHUNK_ENDS.index(s + 1) - 1] \
                            if CHUNK_ENDS.index(s + 1) else 0
                        nc.gpsimd.dma_start(
                            out=out_c[:, c0:s + 1, :],
                            in_=u2all[:, c0:s + 1, :])

                for dd in (u0, u1):
                    dd.pop(t - 4, None)

    _strip_same_engine_waits(nc)
    _fuse_ldweights(nc)
    return nc


def _fuse_ldweights(nc):
    """Convert split LDWEIGHTS+MATMUL pairs back to self-loading matmuls.

    The scheduler splits every 2-byte matmul into an explicit InstLdweights
    followed by the InstMatmult (ldweights=False).  On hardware the pair
    executes serially (~149ns vs 53ns of math for a 128x128 fp16 tile).  The
    split matmul still carries both operands, so dropping the InstLdweights
    and restoring ldweights=None yields the fp32-style self-loading form.
    """
    for fn in nc.m.functions:
        for blk in fn.blocks:
            insts = blk.instructions
            out, pending = [], None
            for inst in insts:
                if type(inst).__name__ == "InstLdweights":
                    assert pending is None
                    pending = inst
                    continue
                if pending is not None:
                    assert type(inst).__name__ == "InstMatmult", inst
                    inst.ldweights = None
                    psi, si = pending.sync_info, inst.sync_info
                    waits = list(psi.on_wait if psi else []) + \
                        list(si.on_wait if si else [])
                    upds = list(psi.on_update if psi else []) + \
                        list(si.on_update if si else [])
                    assert len(waits) <= 1, waits
                    if waits or upds:
                        inst.sync_info = mybir.SyncInfo(
                            on_wait=waits, on_update=upds)
                    pending = None
                out.append(inst)
            assert pending is None
            if len(out) != len(insts):
                blk.instructions = out


_ENG_SEM_PREFIX = {
    mybir.EngineType.Activation: "Activation",
    mybir.EngineType.PE: "PE",
    mybir.EngineType.DVE: "DVE",
    mybir.EngineType.Pool: "Pool",
    mybir.EngineType.SP: "SP",
}


def _strip_same_engine_waits(nc):
    """Drop sem waits an instruction holds on its OWN engine's stream sem.

    Tile emits pool-reuse WAW/WAR hazards as explicit sem waits even when
    producer and consumer share an engine; same-engine execution is in-order
    so such waits are provably satisfied at issue.  Removing them keeps every
    instruction at <=1 sync-wait (a hard limit of this walrus build).
    """
    for fn in nc.m.functions:
        for blk in fn.blocks:
            for inst in blk.instructions:
                si = inst.sync_info
                if si is None or not si.on_wait:
                    continue
                pfx = _ENG_SEM_PREFIX.get(inst.engine)
                if pfx is None:
                    continue
                keep = [w for w in si.on_wait
                        if w.ant_name.rsplit("_", 1)[0] != pfx]
                if len(keep) != len(si.on_wait):
                    inst.sync_info = mybir.SyncInfo(
                        on_wait=keep, on_update=list(si.on_update))
                assert len(keep) <= 1, (
                    f"{inst.name}: {len(keep)} foreign waits remain: "
                    f"{[w.ant_name for w in keep]}")


def _blob(whhT, wihT12):
    """Pack per-core transposed weights into the [P, CW] fp16 blob."""
    b = np.empty((P, CW), np.float16)
    b[:, WHH0:WHH0 + D * K * H] = (
        whhT.reshape(D, K, P, H).transpose(2, 0, 1, 3).reshape(P, D * K * H))
    b[:, WIH0:WIH0 + (D - 1) * K * H] = (
        wihT12.reshape(D - 1, K, P, H).transpose(2, 0, 1, 3)
        .reshape(P, (D - 1) * K * H))
    return b


def _aux(W0, seed, b0, bias12):
    """Arming operands on rows 0..1 of a zero [P, AW] sheet (the zero rows
    let the arming matmuls run at contraction 128)."""
    a = np.zeros((P, AW), np.float16)
    # pre0[h, s] = (W0 @ seed.T + b0)[h]; a[c, s*P+p] = pre0[c*128+p, s]
    pre0 = W0 @ seed.T + b0[:, None]
    a[0:K, A_P0T:A_P0T + S * P] = (
        pre0.reshape(K, P, S).transpose(0, 2, 1).reshape(K, S * P))
    # a[c, (d-1)*P+p] = bias12[d-1, c*128+p]
    a[0:K, A_B12:A_B12 + (D - 1) * P] = (
        bias12.reshape(D - 1, K, P).transpose(1, 0, 2).reshape(K, (D - 1) * P))
    for c in range(K):
        a[c, A_IND + c * T:A_IND + (c + 1) * T] = 1.0
    return a


def kernel(src, trg, Wx_ih, Wx_hh, bx_ih, bx_hh, Wy_ih, Wy_hh, by_ih, by_hh):
    if "nc" not in _cache:
        _cache["nc"] = _build()
    nc = _cache["nc"]

    def tr(w):  # [D,H,H] -> W[d].T contiguous, fp16
        return np.ascontiguousarray(
            np.swapaxes(np.asarray(w, np.float32), 1, 2)).astype(np.float16)

    src = np.asarray(src, np.float32)
    trg = np.asarray(trg, np.float32)
    blob_x = _blob(tr(Wx_hh), tr(Wx_ih)[1:])
    blob_y = _blob(tr(Wy_hh), tr(Wy_ih)[1:])
    bx = np.asarray(bx_ih, np.float32) + np.asarray(bx_hh, np.float32)
    by = np.asarray(by_ih, np.float32) + np.asarray(by_hh, np.float32)
    Wx0 = np.asarray(Wx_ih, np.float32)[0]
    Wy0 = np.asarray(Wy_ih, np.float32)[0]

    in_maps = []
    for b in range(B):  # cores 0-3: x chains
        in_maps.append({"blob": blob_x,
                        "aux": _aux(Wx0, src[b], bx[0], bx[1:])})
    for b in range(B):  # cores 4-7: y chains
        in_maps.append({"blob": blob_y,
                        "aux": _aux(Wy0, trg[b], by[0], by[1:])})

    _cache["last_in_maps"] = in_maps
    globals()["_last_in_maps"] = in_maps
    res = run_bass_kernel_spmd(nc, in_maps, list(range(8)))

    out = np.empty((B, S, T, 2, H), np.float32)
    ii = np.arange(S)[:, None]
    jj = np.arange(T)[None, :]
    idx = (jj - ii) % T  # hx[i,j] = u_i[(j-i)%T]
    for b in range(B):
        # raw core output [p, s, k*T+v] -> [s, H=k*128+p, v]
        arr = np.asarray(res.results[b]["out"], np.float32)
        arr = arr.reshape(P, S, K, T).transpose(1, 2, 0, 3).reshape(S, H, T)
        hx = np.take_along_axis(arr, idx[:, None, :], axis=2)  # [s, H, j]
        out[b, :, :, 0, :] = hx.transpose(0, 2, 1)
        arr = np.asarray(res.results[B + b]["out"], np.float32)
        arr = arr.reshape(P, S, K, T).transpose(1, 2, 0, 3).reshape(S, H, T)
        out[b, :, :, 1, :] = arr.transpose(2, 0, 1)  # [j, H, i] -> [i, j, H]
    return out
